# revision 36
# baseline (speedup 1.0000x reference)
"""Trainium2 Bass kernel for nn_LocalModel (6-encoder local-attention transformer).

Sharding: data-parallel over batch — B=8 batch elements, one per NeuronCore.
Each core runs the full 6-layer encoder stack + final projection for its
batch element on-chip (all weights SBUF-resident in bf16), returning a
[6]-vector; the host gathers them into the [8, 6] output.

Attention uses the zero-masked-softmax identity: with out-of-window scores
set to 0 (not -inf), softmax over the full sequence satisfies
    out_i = (sum_{j in W} (e^{s_ij} - 1) v_j + sum_all v_j)
          / (sum_{j in W} (e^{s_ij} - 1) + S)

Key structure choices vs the f32 baseline:
  * everything bf16 except PSUM accumulation, LN stats and biases
  * key blocks are shifted by -W (=-64): block kb covers keys
    [kb*128-64, kb*128+64), so each 128-query block needs exactly 2 key
    blocks and every score tile is a uniform 256-wide band with ONE mask
    pattern; out-of-range keys are handled by zero-padded K columns
  * the "+sum_all v / +S" terms enter the AV PSUM accumulation as a K=1
    rank-1 matmul against an augmented V-total row (ones column = S)
  * (e^s - 1)*mask is a single fused DVE op (scalar_tensor_tensor)
  * the FFN residual is folded into the fc2 PSUM group via identity matmuls
  * LN rstd = exp(-0.5*ln(var+eps)) so every ACT func (Exp/Ln/Identity/
    Copy/Relu) lives in one activation table -> zero table reloads
  * out_w is prefetched in bf16 at kernel start across 3 DMA queues with a
    partition-contiguous layout; final dot products are fused DVE
    mult+accum ops
"""
import sys
import numpy as np

sys.path.insert(0, "/opt/trn_rl_repo")

B, S, D = 8, 1024, 512
H, Dh, W = 8, 64, 64
HD = 2048           # ffn hidden
C = 6               # classes
ENC = 6
EPS = 1e-5
P = 128
KO = D // P         # 4
HC = HD // P        # 16
NKB = 9             # shifted key blocks
SCALE = Dh ** -0.5

_CACHE = {}
LAST_EXEC_NS = None
LAST_RESULTS = None
TRACE = False


def _build(affine: bool, n_layers: int = ENC, taps: tuple = ()):
    import concourse.bass as bass
    import concourse.tile as tile
    from concourse import bacc, mybir
    from concourse.masks import make_identity

    f32 = mybir.dt.float32
    bf16 = mybir.dt.bfloat16
    AF = mybir.ActivationFunctionType
    OP = mybir.AluOpType

    nc = bacc.Bacc()
    d = {}
    d['xT'] = nc.declare_dram_parameter("xT", [P, KO * S], bf16, isOutput=False)
    for w in ("wqT", "wkT", "wvT"):
        d[w] = nc.declare_dram_parameter(w, [P, KO * D], bf16, isOutput=False)
    for b_ in ("bq", "bk"):
        d[b_] = nc.declare_dram_parameter(b_, [P, KO], f32, isOutput=False)
    d['bv'] = nc.declare_dram_parameter("bv", [D], f32, isOutput=False)
    d['fc1T'] = nc.declare_dram_parameter("fc1T", [P, KO * HD], bf16, isOutput=False)
    d['fc1b'] = nc.declare_dram_parameter("fc1b", [P, HC], f32, isOutput=False)
    d['fc2T'] = nc.declare_dram_parameter("fc2T", [P, HC * D], bf16, isOutput=False)
    d['fc2b'] = nc.declare_dram_parameter("fc2b", [D], f32, isOutput=False)
    d['mask'] = nc.declare_dram_parameter("mask", [P, 512], bf16, isOutput=False)
    d['ident'] = nc.declare_dram_parameter("ident", [P, P], bf16, isOutput=False)
    d['owT'] = nc.declare_dram_parameter("owT", [C, P, KO * S], bf16, isOutput=False)
    if affine:
        d['lng'] = nc.declare_dram_parameter("lng", [D], f32, isOutput=False)
        d['lnb'] = nc.declare_dram_parameter("lnb", [D], f32, isOutput=False)
    out_d = nc.declare_dram_parameter("out", [1, C], f32, isOutput=True)
    tap_d = {}
    for t in taps:
        shapes = {'va': [P, NKB * H * 65], 'q0': [P, S], 'k0': [P, W + S + W],
                  'vtot': [1, H * 65], 'pc0': [P, NKB * 256],
                  'atok': [P, H * D], 'x1T': [P, KO * S],
                  'f0': [P, 8 * D],
                  'xnext': [P, KO * S]}
        tap_d[t] = nc.declare_dram_parameter(
            "tap_" + t, shapes[t], bf16, isOutput=True)

    def bcast_ap(dram_h, parts=P):
        # replicate a [N] dram vector across `parts` partitions
        a = dram_h[:]
        return bass.AP(tensor=a.tensor, offset=a.offset,
                       ap=[[0, parts]] + [list(x) for x in a.ap])

    from contextlib import ExitStack
    with tile.TileContext(nc) as tc, ExitStack() as ctx:
        wpool = ctx.enter_context(tc.tile_pool(name="wpool", bufs=1))
        big = ctx.enter_context(tc.tile_pool(name="big", bufs=3))
        qkp = ctx.enter_context(tc.tile_pool(name="qkp", bufs=1))
        vap = ctx.enter_context(tc.tile_pool(name="vap", bufs=1))
        pcp = ctx.enter_context(tc.tile_pool(name="pcp", bufs=3))
        pp = ctx.enter_context(tc.tile_pool(name="pp", bufs=6))
        atp = ctx.enter_context(tc.tile_pool(name="atp", bufs=1))
        hp = ctx.enter_context(tc.tile_pool(name="hp", bufs=1))
        tmp = ctx.enter_context(tc.tile_pool(name="tmp", bufs=4))
        small = ctx.enter_context(tc.tile_pool(name="small", bufs=8))
        scr = ctx.enter_context(tc.tile_pool(name="scr", bufs=2))
        psP = ctx.enter_context(tc.tile_pool(name="psP", bufs=3, space="PSUM"))
        psS = ctx.enter_context(tc.tile_pool(name="psS", bufs=2, space="PSUM"))
        psV = ctx.enter_context(tc.tile_pool(name="psV", bufs=2, space="PSUM"))
        psT = ctx.enter_context(tc.tile_pool(name="psT", bufs=1, space="PSUM"))

        ident = wpool.tile([P, P], bf16, tag="id")
        nc.sync.dma_start(ident, d['ident'][:])
        ones_row = wpool.tile([1, P], bf16, tag="onr")
        nc.vector.memset(ones_row, 1.0)
        ones_col = wpool.tile([P, 1], bf16, tag="onc")
        nc.vector.memset(ones_col, 1.0)
        # PE clock pre-warm: ~6us of dummy matmuls on resident data while the
        # input DMAs land, so the HAM clock-gate is at 2.4GHz for real work
        for _ in range(28):
            wu = psS.tile([P, 256], f32, tag="s")
            nc.tensor.matmul(wu[:, :P], lhsT=ident, rhs=ident,
                             start=True, stop=True)
            nc.tensor.matmul(wu[:, P:], lhsT=ident, rhs=ident,
                             start=True, stop=True)
        # ---- persistent loads (queues: sync + scalar HWDGE, gpsimd SWDGE) ----
        xT = big.tile([P, KO, S], bf16, tag="big")
        nc.sync.dma_start(xT[:, 0:2, :], d['xT'][:, 0:2 * S])
        nc.scalar.dma_start(xT[:, 2:4, :], d['xT'][:, 2 * S:4 * S])
        wq_sb = wpool.tile([P, KO, D], bf16, tag="wq")
        wk_sb = wpool.tile([P, KO, D], bf16, tag="wk")
        wv_sb = wpool.tile([P, KO, D], bf16, tag="wv")
        nc.sync.dma_start(wq_sb, d['wqT'][:])
        nc.scalar.dma_start(wk_sb, d['wkT'][:])
        nc.scalar.dma_start(wv_sb, d['wvT'][:])
        fc1_sb = wpool.tile([P, KO, HD], bf16, tag="fc1")
        nc.sync.dma_start(fc1_sb, d['fc1T'][:])
        fc2_sb = wpool.tile([P, HC, D], bf16, tag="fc2")
        nc.sync.dma_start(fc2_sb, d['fc2T'][:])
        bq_sb = wpool.tile([P, KO], f32, tag="bq")
        bk_sb = wpool.tile([P, KO], f32, tag="bk")
        nc.gpsimd.dma_start(bq_sb, d['bq'][:])
        nc.gpsimd.dma_start(bk_sb, d['bk'][:])
        bv_bc = wpool.tile([P, D], f32, tag="bv")
        nc.gpsimd.dma_start(out=bv_bc, in_=bcast_ap(d['bv']))
        fc1b_sb = wpool.tile([P, HC], f32, tag="fc1b")
        nc.gpsimd.dma_start(fc1b_sb, d['fc1b'][:])
        fc2b_bc = wpool.tile([P, D], f32, tag="fc2b")
        nc.gpsimd.dma_start(out=fc2b_bc, in_=bcast_ap(d['fc2b']))
        mask_sb = wpool.tile([P, 512], bf16, tag="mask")
        nc.gpsimd.dma_start(mask_sb, d['mask'][:])
        if affine:
            g_bc = wpool.tile([P, D], f32, tag="g")
            b_bc = wpool.tile([P, D], f32, tag="b")
            nc.gpsimd.dma_start(out=g_bc, in_=bcast_ap(d['lng']))
            nc.gpsimd.dma_start(out=b_bc, in_=bcast_ap(d['lnb']))
        # out_w prefetch: fills DMA queues while layers 1-2 compute
        owt = []
        qs_cycle = [nc.sync, nc.scalar]
        for c in range(C):
            t = wpool.tile([P, KO, S], bf16, tag=f"ow{c}")
            qs_cycle[c % 2].dma_start(t, d['owT'][c])
            owt.append(t)

        eps_sb = wpool.tile([P, 1], f32, tag="eps")
        nc.vector.memset(eps_sb, EPS)
        vtot_sb = wpool.tile([1, H * 65], bf16, tag="vtot")
        bv1k = wpool.tile([1, D], f32, tag="bv1k")
        nc.scalar.mul(out=bv1k, in_=bv_bc[0:1, :], mul=float(S))

        def layer_norm_apply(src_ap, out_tile):
            """LayerNorm src [P,512] -> out_tile [P,512] bf16 (token-major).
            rstd via exp(-0.5*ln(var+eps)) to stay in one ACT table."""
            st = small.tile([P, 6], f32, tag="st")
            mv = small.tile([P, 2], f32, tag="mv")
            nc.vector.bn_stats(out=st, in_=src_ap)
            nc.vector.bn_aggr(out=mv, in_=st)
            rstd = small.tile([P, 1], f32, tag="rs")
            nc.scalar.activation(out=rstd, in_=mv[:, 1:2], func=AF.Sqrt,
                                 bias=eps_sb[:, 0:1])
            nc.vector.reciprocal(out=rstd, in_=rstd)
            nc.vector.tensor_scalar(out=out_tile, in0=src_ap,
                                    scalar1=mv[:, 0:1], scalar2=rstd,
                                    op0=OP.subtract, op1=OP.mult)
            if affine:
                nc.vector.tensor_tensor(out=out_tile, in0=out_tile, in1=g_bc,
                                        op=OP.mult)
                nc.vector.tensor_tensor(out=out_tile, in0=out_tile, in1=b_bc,
                                        op=OP.add)

        def transpose_to(src_tile, dst_tile, tb):
            """src [P, 512] bf16 token-major block tb -> dst [P, KO, S]."""
            pt = psS.tile([P, D], bf16, tag="s")
            for dc in range(KO):
                nc.tensor.transpose(pt[:, dc * P:(dc + 1) * P],
                                    src_tile[:, dc * P:(dc + 1) * P], ident)
            for dc in range(KO):
                if dc < 2:
                    nc.scalar.copy(
                        out=dst_tile[:, dc, tb * P:(tb + 1) * P],
                        in_=pt[:, dc * P:(dc + 1) * P])
                else:
                    nc.vector.tensor_scalar_add(
                        out=dst_tile[:, dc, tb * P:(tb + 1) * P],
                        in0=pt[:, dc * P:(dc + 1) * P], scalar1=0.0)

        for L in range(n_layers):
            # ---------- Q/K projections (feature-major) ----------
            q_t, k_t = [], []
            for mc in range(KO):
                qm = qkp.tile([P, S], bf16, tag=f"q{mc}")
                km = qkp.tile([P, W + S + W], bf16, tag=f"k{mc}")
                q_t.append(qm)
                k_t.append(km)
                nc.vector.memset(km[:, 0:W], 0.0)
                nc.vector.memset(km[:, W + S:], 0.0)
                for half in range(2):
                    cs = slice(half * 512, (half + 1) * 512)
                    pq = psP.tile([P, D], f32, tag="pj")
                    for ko in range(KO):
                        nc.tensor.matmul(
                            pq, lhsT=wq_sb[:, ko, mc * P:(mc + 1) * P],
                            rhs=xT[:, ko, cs],
                            start=(ko == 0), stop=(ko == KO - 1))
                    nc.scalar.activation(out=qm[:, cs], in_=pq,
                                         func=AF.Identity, bias=bq_sb[:, mc:mc + 1])
                    pk = psP.tile([P, D], f32, tag="pj")
                    for ko in range(KO):
                        nc.tensor.matmul(
                            pk, lhsT=wk_sb[:, ko, mc * P:(mc + 1) * P],
                            rhs=xT[:, ko, cs],
                            start=(ko == 0), stop=(ko == KO - 1))
                    nc.vector.tensor_scalar_add(
                        out=km[:, W + half * 512:W + (half + 1) * 512], in0=pk,
                        scalar1=bk_sb[:, mc:mc + 1])

            # ---------- V totals part 1: xsum (DVE, overlaps QKV matmuls) ----
            xs32 = small.tile([P, KO], f32, tag="xs")
            for ko in range(KO):
                nc.vector.reduce_sum(out=xs32[:, ko:ko + 1], in_=xT[:, ko, :],
                                     axis=mybir.AxisListType.X)

            # ---------- V projection into shifted key blocks ----------
            va = vap.tile([P, NKB, H, 65], bf16, tag="va")
            # edge blocks: zero the never-written halves (incl. ones col),
            # then set all ones-columns
            nc.vector.memset(va[0:64, 0, :, :], 0.0)
            nc.vector.memset(va[64:P, NKB - 1, :, :], 0.0)
            nc.vector.memset(va[:, :, :, 64:65], 1.0)
            for kb in range(NKB):
                tok0 = kb * P - W
                t0, t1 = max(0, tok0), min(S, tok0 + P)
                po, width = t0 - tok0, t1 - t0
                pv = psP.tile([P, D], f32, tag="pj")
                for ko in range(KO):
                    nc.tensor.matmul(
                        pv[po:po + width, :], lhsT=xT[:, ko, t0:t1],
                        rhs=wv_sb[:, ko, :],
                        start=(ko == 0), stop=(ko == KO - 1))
                nc.vector.tensor_tensor(
                    out=va[po:po + width, kb, :, 0:64],
                    in0=pv[po:po + width, :].rearrange("p (h a) -> p h a", a=64),
                    in1=bv_bc[po:po + width, :].rearrange("p (h a) -> p h a", a=64),
                    op=OP.add)

            # ---------- V totals part 2: (sum_t x) @ wvT + S*bv ----------
            xsr = small.tile([P, KO], bf16, tag="xsr")
            nc.scalar.copy(out=xsr, in_=xs32)
            pvt = psT.tile([1, D], f32, tag="vt")
            for ko in range(KO):
                nc.tensor.matmul(pvt, lhsT=xsr[:, ko:ko + 1],
                                 rhs=wv_sb[:, ko, :],
                                 start=(ko == 0), stop=(ko == KO - 1))
            nc.vector.tensor_tensor(
                out=vtot_sb.rearrange("p (h a) -> p h a", a=65)[:, :, 0:64],
                in0=pvt.rearrange("p (h a) -> p h a", a=64),
                in1=bv1k.rearrange("p (h a) -> p h a", a=64), op=OP.add)
            nc.vector.memset(
                vtot_sb.rearrange("p (h a) -> p h a", a=65)[:, :, 64:65],
                float(S))

            if L == 0 and 'va' in tap_d:
                nc.sync.dma_start(tap_d['va'][:], va)
            if L == 0 and 'q0' in tap_d:
                nc.sync.dma_start(tap_d['q0'][:], q_t[0])
            if L == 0 and 'k0' in tap_d:
                nc.sync.dma_start(tap_d['k0'][:], k_t[0])
            if L == 0 and 'vtot' in tap_d:
                nc.sync.dma_start(tap_d['vtot'][:], vtot_sb)
            # ---------- attention (shifted key blocks) ----------
            # software-pipelined: AV of head h-1 is emitted after the scores
            # of head h, so the PE queue never head-blocks on exp/mask
            a_tok = atp.tile([P, H, D], bf16, tag="at")

            def scores_block(h):
                hko = h // 2
                hr = slice(64 * (h % 2), 64 * (h % 2) + 64)
                pc = pcp.tile([P, NKB, 256], bf16, tag="pc")
                for kb in range(NKB):
                    qlo = max(0, (kb - 1) * P)
                    qhi = min(S, (kb + 1) * P)
                    qw = qhi - qlo
                    ps = psS.tile([P, 256], f32, tag="s")
                    nc.tensor.matmul(
                        ps[:, :qw],
                        lhsT=k_t[hko][hr, kb * P:(kb + 1) * P],
                        rhs=q_t[hko][hr, qlo:qhi],
                        start=True, stop=True)
                    es = pp.tile([P, 256], bf16, tag="es")
                    nc.scalar.activation(out=es[:, :qw], in_=ps[:, :qw],
                                         func=AF.Exp, scale=SCALE)
                    mc0 = 128 if kb == 0 else 0
                    nc.vector.scalar_tensor_tensor(
                        out=pc[:, kb, :qw], in0=es[:, :qw], scalar=1.0,
                        in1=mask_sb[:, mc0:mc0 + qw],
                        op0=OP.subtract, op1=OP.mult)
                if L == 0 and h == 0 and 'pc0' in tap_d:
                    nc.sync.dma_start(tap_d['pc0'][:], pc)
                return pc

            def av_block(h, pc, post_qb=None):
                for qb in range(8):
                    pav = psV.tile([P, 65], f32, tag="av")
                    c0 = 0 if qb == 0 else 128
                    nc.tensor.matmul(pav, lhsT=pc[:, qb, c0:c0 + P],
                                     rhs=va[:, qb, h, :], start=True, stop=False)
                    nc.tensor.matmul(pav, lhsT=pc[:, qb + 1, 0:P],
                                     rhs=va[:, qb + 1, h, :], start=False,
                                     stop=False)
                    nc.tensor.matmul(pav, lhsT=ones_row,
                                     rhs=vtot_sb[0:1, h * 65:(h + 1) * 65],
                                     start=False, stop=True)
                    rc = small.tile([P, 1], f32, tag="rc")
                    nc.vector.reciprocal(out=rc, in_=pav[:, 64:65])
                    nc.vector.tensor_scalar_mul(
                        out=a_tok[:, qb, h * 64:(h + 1) * 64],
                        in0=pav[:, 0:64], scalar1=rc)
                    if post_qb is not None:
                        post_qb(qb)

            x1T = big.tile([P, KO, S], bf16, tag="big")

            def ln1_block(qb):
                xn = tmp.tile([P, D], bf16, tag="xn")
                layer_norm_apply(a_tok[:, qb, :], xn)
                transpose_to(xn, x1T, qb)

            pend = []
            for h in range(H):
                pend.append((h, scores_block(h)))
                if len(pend) > 2:
                    av_block(*pend.pop(0))
            for item in pend:
                av_block(*item)
            for qb in range(8):
                ln1_block(qb)

            if L == 0 and 'atok' in tap_d:
                nc.sync.dma_start(tap_d['atok'][:], a_tok)
            if L == 0 and 'x1T' in tap_d:
                nc.sync.dma_start(tap_d['x1T'][:], x1T)
            # ---------- FFN + residual + LN2 -> next xT ----------
            xT_next = big.tile([P, KO, S], bf16, tag="big")
            for tq in range(2):
                qs = slice(tq * 512, (tq + 1) * 512)
                hts = []
                for hc in range(HC):
                    ph = psP.tile([P, D], f32, tag="pj")
                    for ko in range(KO):
                        nc.tensor.matmul(
                            ph,
                            lhsT=fc1_sb[:, ko, hc * P:(hc + 1) * P],
                            rhs=x1T[:, ko, qs],
                            start=(ko == 0), stop=(ko == KO - 1))
                    ht = hp.tile([P, D], bf16, tag=f"h{hc}")
                    if hc % 2 == 0:
                        nc.scalar.activation(out=ht, in_=ph, func=AF.Relu,
                                             bias=fc1b_sb[:, hc:hc + 1])
                    else:
                        nc.vector.tensor_scalar(
                            out=ht, in0=ph, scalar1=fc1b_sb[:, hc:hc + 1],
                            scalar2=0.0, op0=OP.add, op1=OP.max)
                    hts.append(ht)
                for tb2 in range(4):
                    tb = tq * 4 + tb2
                    pf = psP.tile([P, D], f32, tag="pj")
                    # fc2 first (start=True on hc0 clears the bank's
                    # has_written flags bank-wide), then the residual x1
                    # (feature-major, un-transposed) accumulates via
                    # identity matmuls with start=False
                    for hc in range(HC):
                        nc.tensor.matmul(
                            pf, lhsT=hts[hc][:, tb2 * P:(tb2 + 1) * P],
                            rhs=fc2_sb[:, hc, :],
                            start=(hc == 0), stop=False)
                    for dc in range(KO):
                        nc.tensor.matmul(
                            pf[:, dc * P:(dc + 1) * P],
                            lhsT=x1T[:, dc, tb * P:(tb + 1) * P],
                            rhs=ident, start=False, stop=(dc == KO - 1))
                    f = tmp.tile([P, D], bf16, tag="xn")
                    nc.vector.scalar_tensor_tensor(
                        out=f, in0=pf, scalar=0.0, in1=fc2b_bc,
                        op0=OP.add, op1=OP.add)
                    if L == 0 and 'f0' in tap_d:
                        nc.sync.dma_start(
                            tap_d['f0'][:, tb * D:(tb + 1) * D], f)
                    xn2 = tmp.tile([P, D], bf16, tag="xn")
                    layer_norm_apply(f, xn2)
                    transpose_to(xn2, xT_next, tb)
            if L == 0 and 'xnext' in tap_d:
                nc.sync.dma_start(tap_d['xnext'][:], xT_next)
            xT = xT_next

        # ---------- final projection: out[r] = sum(xT * owT[r]) ----------
        # DVE elementwise product (bf16, 4x mode), PE column-sum via
        # ones-vector matmuls accumulating 8 chunks into [1,512], then a
        # single-lane DVE reduce per class.
        red1 = wpool.tile([1, C], f32, tag="red1")
        for r in range(C):
            pcs_ = psT.tile([1, D], f32, tag="vt")
            for ko in range(KO):
                sc = scr.tile([P, S], bf16, tag="sc")
                nc.vector.tensor_tensor(
                    out=sc, in0=xT[:, ko, :], in1=owt[r][:, ko, :], op=OP.mult)
                for hf in range(2):
                    nc.tensor.matmul(
                        pcs_, lhsT=ones_col,
                        rhs=sc[:, hf * 512:(hf + 1) * 512],
                        start=(ko == 0 and hf == 0),
                        stop=(ko == KO - 1 and hf == 1))
            osc = scr.tile([1, D], f32, tag="osc")
            nc.scalar.copy(out=osc, in_=pcs_)
            nc.vector.reduce_sum(out=red1[:, r:r + 1], in_=osc,
                                 axis=mybir.AxisListType.X)
        nc.sync.dma_start(out_d[:], red1)

    nc.compile()
    return nc


def _shuffle_pko(a, inner):
    """[D_out*?, inner] row-major -> [P, blocks*inner] partition-major."""
    n = a.shape[0] // P
    return np.ascontiguousarray(
        a.reshape(n, P, inner).transpose(1, 0, 2).reshape(P, n * inner))


def _prep(inputs):
    """Host-side input prep shared across cores."""
    import ml_dtypes
    bf = ml_dtypes.bfloat16
    emb = np.asarray(inputs['emb'], dtype=np.float32)
    idx = np.asarray(inputs['inputs'])
    pos = np.arange(S, dtype=np.float32)[:, None]
    div = np.exp(-np.log(10000.0) * np.arange(0, D, 2, dtype=np.float32) / D)
    ang = pos * div
    pe = np.zeros((S, D), dtype=np.float32)
    pe[:, 0::2] = np.sin(ang)
    pe[:, 1::2] = np.cos(ang)
    x0 = emb[idx] + pe[None]  # [B, S, D]

    # mask[p, c] = 1 iff p < c <= p+128 (shifted-block band), tiled twice so
    # batched kb-pair ops can use one contiguous [P,512] operand
    jj = np.arange(P)[:, None]
    cc = np.arange(256)[None, :]
    m1 = ((cc > jj) & (cc <= jj + P))
    mask = np.concatenate([m1, m1], axis=1).astype(bf)

    ln_g = np.asarray(inputs['ln_g'], dtype=np.float32)
    ln_b = np.asarray(inputs['ln_b'], dtype=np.float32)
    affine = not (np.all(ln_g == 1.0) and np.all(ln_b == 0.0))

    out_w = np.asarray(inputs['out_w'], dtype=np.float32)
    # owT[c][p, ko*S+n] = out_w[c, n*D + ko*128+p]
    owT = out_w.reshape(C, S, D).transpose(0, 2, 1)  # [C, D, S]
    owT = np.ascontiguousarray(
        owT.reshape(C, KO, P, S).transpose(0, 2, 1, 3)
        .reshape(C, P, KO * S)).astype(bf)

    common = {
        'wqT': _shuffle_pko(np.asarray(inputs['wq'], np.float32).T.astype(bf), D),
        'wkT': _shuffle_pko(np.asarray(inputs['wk'], np.float32).T.astype(bf), D),
        'wvT': _shuffle_pko(np.asarray(inputs['wv'], np.float32).T.astype(bf), D),
        'bq': np.ascontiguousarray(
            np.asarray(inputs['bq'], np.float32).reshape(KO, P).T),
        'bk': np.ascontiguousarray(
            np.asarray(inputs['bk'], np.float32).reshape(KO, P).T),
        'bv': np.ascontiguousarray(np.asarray(inputs['bv'], np.float32)),
        'fc1T': _shuffle_pko(
            np.asarray(inputs['fc1_w'], np.float32).T.astype(bf), HD),
        'fc1b': np.ascontiguousarray(
            np.asarray(inputs['fc1_b'], np.float32).reshape(HC, P).T),
        'fc2T': _shuffle_pko(
            np.asarray(inputs['fc2_w'], np.float32).T.astype(bf), D),
        'fc2b': np.ascontiguousarray(np.asarray(inputs['fc2_b'], np.float32)),
        'mask': mask,
        'ident': np.eye(P, dtype=bf),
        'owT': owT,
    }
    if affine:
        common['lng'] = np.ascontiguousarray(ln_g)
        common['lnb'] = np.ascontiguousarray(ln_b)
    per_core = [
        {'xT': _shuffle_pko(x0[b].T.astype(bf), S)}
        for b in range(B)
    ]
    return common, per_core, affine


def kernel(**inputs):
    global LAST_EXEC_NS, LAST_RESULTS
    from concourse.bass_utils import run_bass_kernel_spmd

    common, per_core, affine = _prep(inputs)
    if affine not in _CACHE:
        _CACHE[affine] = _build(affine)
    nc = _CACHE[affine]

    in_maps = [dict(common, **pc) for pc in per_core]
    res = run_bass_kernel_spmd(nc, in_maps, list(range(B)), trace=TRACE)
    LAST_EXEC_NS = res.exec_time_ns
    LAST_RESULTS = res
    out = np.stack([res.results[b]["out"][0] for b in range(B)], axis=0)
    out = out + np.asarray(inputs['out_b'], np.float32)[None, :]
    return out.astype(np.float32)


# revision 37
# speedup vs baseline: 1.0088x; 1.0088x over previous
"""Trainium2 Bass kernel for nn_LocalModel (6-encoder local-attention transformer).

Sharding: data-parallel over batch — B=8 batch elements, one per NeuronCore.
Each core runs the full 6-layer encoder stack + final projection for its
batch element on-chip (all weights SBUF-resident in bf16), returning a
[6]-vector; the host gathers them into the [8, 6] output.

Attention uses the zero-masked-softmax identity: with out-of-window scores
set to 0 (not -inf), softmax over the full sequence satisfies
    out_i = (sum_{j in W} (e^{s_ij} - 1) v_j + sum_all v_j)
          / (sum_{j in W} (e^{s_ij} - 1) + S)

Key structure choices vs the f32 baseline:
  * everything bf16 except PSUM accumulation, LN stats and biases
  * key blocks are shifted by -W (=-64): block kb covers keys
    [kb*128-64, kb*128+64), so each 128-query block needs exactly 2 key
    blocks and every score tile is a uniform 256-wide band with ONE mask
    pattern; out-of-range keys are handled by zero-padded K columns
  * the "+sum_all v / +S" terms enter the AV PSUM accumulation as a K=1
    rank-1 matmul against an augmented V-total row (ones column = S)
  * (e^s - 1)*mask is a single fused DVE op (scalar_tensor_tensor)
  * the FFN residual is folded into the fc2 PSUM group via identity matmuls
  * LN rstd = exp(-0.5*ln(var+eps)) so every ACT func (Exp/Ln/Identity/
    Copy/Relu) lives in one activation table -> zero table reloads
  * out_w is prefetched in bf16 at kernel start across 3 DMA queues with a
    partition-contiguous layout; final dot products are fused DVE
    mult+accum ops
"""
import sys
import numpy as np

sys.path.insert(0, "/opt/trn_rl_repo")

B, S, D = 8, 1024, 512
H, Dh, W = 8, 64, 64
HD = 2048           # ffn hidden
C = 6               # classes
ENC = 6
EPS = 1e-5
P = 128
KO = D // P         # 4
HC = HD // P        # 16
NKB = 9             # shifted key blocks
SCALE = Dh ** -0.5

_CACHE = {}
LAST_EXEC_NS = None
LAST_RESULTS = None
TRACE = False


def _build(affine: bool, n_layers: int = ENC, taps: tuple = ()):
    import concourse.bass as bass
    import concourse.tile as tile
    from concourse import bacc, mybir
    from concourse.masks import make_identity

    f32 = mybir.dt.float32
    bf16 = mybir.dt.bfloat16
    AF = mybir.ActivationFunctionType
    OP = mybir.AluOpType

    nc = bacc.Bacc()
    d = {}
    d['xT'] = nc.declare_dram_parameter("xT", [P, KO * S], bf16, isOutput=False)
    for w in ("wqT", "wkT", "wvT"):
        d[w] = nc.declare_dram_parameter(w, [P, KO * D], bf16, isOutput=False)
    for b_ in ("bq", "bk"):
        d[b_] = nc.declare_dram_parameter(b_, [P, KO], f32, isOutput=False)
    d['bv'] = nc.declare_dram_parameter("bv", [D], f32, isOutput=False)
    d['fc1T'] = nc.declare_dram_parameter("fc1T", [P, KO * HD], bf16, isOutput=False)
    d['fc1b'] = nc.declare_dram_parameter("fc1b", [P, HC], f32, isOutput=False)
    d['fc2T'] = nc.declare_dram_parameter("fc2T", [P, HC * D], bf16, isOutput=False)
    d['fc2b'] = nc.declare_dram_parameter("fc2b", [D], f32, isOutput=False)
    d['mask'] = nc.declare_dram_parameter("mask", [P, 512], bf16, isOutput=False)
    d['ident'] = nc.declare_dram_parameter("ident", [P, P], bf16, isOutput=False)
    d['owT'] = nc.declare_dram_parameter("owT", [C, P, KO * S], bf16, isOutput=False)
    if affine:
        d['lng'] = nc.declare_dram_parameter("lng", [D], f32, isOutput=False)
        d['lnb'] = nc.declare_dram_parameter("lnb", [D], f32, isOutput=False)
    out_d = nc.declare_dram_parameter("out", [1, C], f32, isOutput=True)
    tap_d = {}
    for t in taps:
        shapes = {'va': [P, NKB * H * 65], 'q0': [P, S], 'k0': [P, W + S + W],
                  'vtot': [1, H * 65], 'pc0': [P, NKB * 256],
                  'atok': [P, H * D], 'x1T': [P, KO * S],
                  'f0': [P, 8 * D],
                  'xnext': [P, KO * S]}
        tap_d[t] = nc.declare_dram_parameter(
            "tap_" + t, shapes[t], bf16, isOutput=True)

    def bcast_ap(dram_h, parts=P):
        # replicate a [N] dram vector across `parts` partitions
        a = dram_h[:]
        return bass.AP(tensor=a.tensor, offset=a.offset,
                       ap=[[0, parts]] + [list(x) for x in a.ap])

    from contextlib import ExitStack
    with tile.TileContext(nc) as tc, ExitStack() as ctx:
        wpool = ctx.enter_context(tc.tile_pool(name="wpool", bufs=1))
        big = ctx.enter_context(tc.tile_pool(name="big", bufs=3))
        qkp = ctx.enter_context(tc.tile_pool(name="qkp", bufs=1))
        vap = ctx.enter_context(tc.tile_pool(name="vap", bufs=1))
        pcp = ctx.enter_context(tc.tile_pool(name="pcp", bufs=3))
        pp = ctx.enter_context(tc.tile_pool(name="pp", bufs=6))
        atp = ctx.enter_context(tc.tile_pool(name="atp", bufs=1))
        hp = ctx.enter_context(tc.tile_pool(name="hp", bufs=1))
        tmp = ctx.enter_context(tc.tile_pool(name="tmp", bufs=4))
        small = ctx.enter_context(tc.tile_pool(name="small", bufs=8))
        scr = ctx.enter_context(tc.tile_pool(name="scr", bufs=2))
        psP = ctx.enter_context(tc.tile_pool(name="psP", bufs=3, space="PSUM"))
        psS = ctx.enter_context(tc.tile_pool(name="psS", bufs=2, space="PSUM"))
        psV = ctx.enter_context(tc.tile_pool(name="psV", bufs=2, space="PSUM"))
        psT = ctx.enter_context(tc.tile_pool(name="psT", bufs=1, space="PSUM"))

        ident = wpool.tile([P, P], bf16, tag="id")
        nc.sync.dma_start(ident, d['ident'][:])
        ones_row = wpool.tile([1, P], bf16, tag="onr")
        nc.vector.memset(ones_row, 1.0)
        ones_col = wpool.tile([P, 1], bf16, tag="onc")
        nc.vector.memset(ones_col, 1.0)
        # PE clock pre-warm: ~6us of dummy matmuls on resident data while the
        # input DMAs land, so the HAM clock-gate is at 2.4GHz for real work
        for _ in range(28):
            wu = psS.tile([P, 256], f32, tag="s")
            nc.tensor.matmul(wu[:, :P], lhsT=ident, rhs=ident,
                             start=True, stop=True)
            nc.tensor.matmul(wu[:, P:], lhsT=ident, rhs=ident,
                             start=True, stop=True)
        # ---- persistent loads (queues: sync + scalar HWDGE, gpsimd SWDGE) ----
        xT = big.tile([P, KO, S], bf16, tag="big")
        nc.sync.dma_start(xT[:, 0:2, :], d['xT'][:, 0:2 * S])
        nc.scalar.dma_start(xT[:, 2:4, :], d['xT'][:, 2 * S:4 * S])
        wq_sb = wpool.tile([P, KO, D], bf16, tag="wq")
        wk_sb = wpool.tile([P, KO, D], bf16, tag="wk")
        wv_sb = wpool.tile([P, KO, D], bf16, tag="wv")
        nc.sync.dma_start(wq_sb, d['wqT'][:])
        nc.scalar.dma_start(wk_sb, d['wkT'][:])
        nc.scalar.dma_start(wv_sb, d['wvT'][:])
        fc1_sb = wpool.tile([P, KO, HD], bf16, tag="fc1")
        nc.sync.dma_start(fc1_sb, d['fc1T'][:])
        fc2_sb = wpool.tile([P, HC, D], bf16, tag="fc2")
        nc.sync.dma_start(fc2_sb, d['fc2T'][:])
        bq_sb = wpool.tile([P, KO], f32, tag="bq")
        bk_sb = wpool.tile([P, KO], f32, tag="bk")
        nc.gpsimd.dma_start(bq_sb, d['bq'][:])
        nc.gpsimd.dma_start(bk_sb, d['bk'][:])
        bv_bc = wpool.tile([P, D], f32, tag="bv")
        nc.gpsimd.dma_start(out=bv_bc, in_=bcast_ap(d['bv']))
        fc1b_sb = wpool.tile([P, HC], f32, tag="fc1b")
        nc.gpsimd.dma_start(fc1b_sb, d['fc1b'][:])
        fc2b_bc = wpool.tile([P, D], f32, tag="fc2b")
        nc.gpsimd.dma_start(out=fc2b_bc, in_=bcast_ap(d['fc2b']))
        mask_sb = wpool.tile([P, 512], bf16, tag="mask")
        nc.gpsimd.dma_start(mask_sb, d['mask'][:])
        if affine:
            g_bc = wpool.tile([P, D], f32, tag="g")
            b_bc = wpool.tile([P, D], f32, tag="b")
            nc.gpsimd.dma_start(out=g_bc, in_=bcast_ap(d['lng']))
            nc.gpsimd.dma_start(out=b_bc, in_=bcast_ap(d['lnb']))
        # out_w prefetch: fills DMA queues while layers 1-2 compute
        owt = []
        qs_cycle = [nc.sync, nc.scalar]
        for c in range(C):
            t = wpool.tile([P, KO, S], bf16, tag=f"ow{c}")
            qs_cycle[c % 2].dma_start(t, d['owT'][c])
            owt.append(t)

        eps_sb = wpool.tile([P, 1], f32, tag="eps")
        nc.vector.memset(eps_sb, EPS)
        vtot_sb = wpool.tile([1, H * 65], bf16, tag="vtot")
        bv1k = wpool.tile([1, D], f32, tag="bv1k")
        nc.scalar.mul(out=bv1k, in_=bv_bc[0:1, :], mul=float(S))

        def layer_norm_apply(src_ap, out_tile):
            """LayerNorm src [P,512] -> out_tile [P,512] bf16 (token-major).
            rstd via exp(-0.5*ln(var+eps)) to stay in one ACT table."""
            st = small.tile([P, 6], f32, tag="st")
            mv = small.tile([P, 2], f32, tag="mv")
            nc.vector.bn_stats(out=st, in_=src_ap)
            nc.vector.bn_aggr(out=mv, in_=st)
            rstd = small.tile([P, 1], f32, tag="rs")
            nc.scalar.activation(out=rstd, in_=mv[:, 1:2], func=AF.Sqrt,
                                 bias=eps_sb[:, 0:1])
            nc.vector.reciprocal(out=rstd, in_=rstd)
            nc.vector.tensor_scalar(out=out_tile, in0=src_ap,
                                    scalar1=mv[:, 0:1], scalar2=rstd,
                                    op0=OP.subtract, op1=OP.mult)
            if affine:
                nc.vector.tensor_tensor(out=out_tile, in0=out_tile, in1=g_bc,
                                        op=OP.mult)
                nc.vector.tensor_tensor(out=out_tile, in0=out_tile, in1=b_bc,
                                        op=OP.add)

        def transpose_to(src_tile, dst_tile, tb):
            """src [P, 512] bf16 token-major block tb -> dst [P, KO, S]."""
            pt = psS.tile([P, D], bf16, tag="s")
            for dc in range(KO):
                nc.tensor.transpose(pt[:, dc * P:(dc + 1) * P],
                                    src_tile[:, dc * P:(dc + 1) * P], ident)
            for dc in range(KO):
                if dc < 2:
                    nc.scalar.copy(
                        out=dst_tile[:, dc, tb * P:(tb + 1) * P],
                        in_=pt[:, dc * P:(dc + 1) * P])
                else:
                    nc.vector.tensor_scalar_add(
                        out=dst_tile[:, dc, tb * P:(tb + 1) * P],
                        in0=pt[:, dc * P:(dc + 1) * P], scalar1=0.0)

        for L in range(n_layers):
            # ---------- Q/K projections (feature-major) ----------
            q_t, k_t = [], []
            for mc in range(KO):
                qm = qkp.tile([P, S], bf16, tag=f"q{mc}")
                km = qkp.tile([P, W + S + W], bf16, tag=f"k{mc}")
                q_t.append(qm)
                k_t.append(km)
                nc.vector.memset(km[:, 0:W], 0.0)
                nc.vector.memset(km[:, W + S:], 0.0)
                for half in range(2):
                    cs = slice(half * 512, (half + 1) * 512)
                    pq = psP.tile([P, D], f32, tag="pj")
                    for ko in range(KO):
                        nc.tensor.matmul(
                            pq, lhsT=wq_sb[:, ko, mc * P:(mc + 1) * P],
                            rhs=xT[:, ko, cs],
                            start=(ko == 0), stop=(ko == KO - 1))
                    nc.scalar.activation(out=qm[:, cs], in_=pq,
                                         func=AF.Identity, bias=bq_sb[:, mc:mc + 1])
                    pk = psP.tile([P, D], f32, tag="pj")
                    for ko in range(KO):
                        nc.tensor.matmul(
                            pk, lhsT=wk_sb[:, ko, mc * P:(mc + 1) * P],
                            rhs=xT[:, ko, cs],
                            start=(ko == 0), stop=(ko == KO - 1))
                    nc.vector.tensor_scalar_add(
                        out=km[:, W + half * 512:W + (half + 1) * 512], in0=pk,
                        scalar1=bk_sb[:, mc:mc + 1])

            # ---------- V totals part 1: xsum (DVE, overlaps QKV matmuls) ----
            xs32 = small.tile([P, KO], f32, tag="xs")
            for ko in range(KO):
                nc.vector.reduce_sum(out=xs32[:, ko:ko + 1], in_=xT[:, ko, :],
                                     axis=mybir.AxisListType.X)

            # ---------- V projection into shifted key blocks ----------
            va = vap.tile([P, NKB, H, 65], bf16, tag="va")
            # edge blocks: zero the never-written halves (incl. ones col),
            # then set all ones-columns
            nc.vector.memset(va[0:64, 0, :, :], 0.0)
            nc.vector.memset(va[64:P, NKB - 1, :, :], 0.0)
            nc.vector.memset(va[:, :, :, 64:65], 1.0)
            for kb in range(NKB):
                tok0 = kb * P - W
                t0, t1 = max(0, tok0), min(S, tok0 + P)
                po, width = t0 - tok0, t1 - t0
                pv = psP.tile([P, D], f32, tag="pj")
                for ko in range(KO):
                    nc.tensor.matmul(
                        pv[po:po + width, :], lhsT=xT[:, ko, t0:t1],
                        rhs=wv_sb[:, ko, :],
                        start=(ko == 0), stop=(ko == KO - 1))
                nc.vector.tensor_tensor(
                    out=va[po:po + width, kb, :, 0:64],
                    in0=pv[po:po + width, :].rearrange("p (h a) -> p h a", a=64),
                    in1=bv_bc[po:po + width, :].rearrange("p (h a) -> p h a", a=64),
                    op=OP.add)

            # ---------- V totals part 2: (sum_t x) @ wvT + S*bv ----------
            xsr = small.tile([P, KO], bf16, tag="xsr")
            nc.scalar.copy(out=xsr, in_=xs32)
            pvt = psT.tile([1, D], f32, tag="vt")
            for ko in range(KO):
                nc.tensor.matmul(pvt, lhsT=xsr[:, ko:ko + 1],
                                 rhs=wv_sb[:, ko, :],
                                 start=(ko == 0), stop=(ko == KO - 1))
            nc.vector.tensor_tensor(
                out=vtot_sb.rearrange("p (h a) -> p h a", a=65)[:, :, 0:64],
                in0=pvt.rearrange("p (h a) -> p h a", a=64),
                in1=bv1k.rearrange("p (h a) -> p h a", a=64), op=OP.add)
            nc.vector.memset(
                vtot_sb.rearrange("p (h a) -> p h a", a=65)[:, :, 64:65],
                float(S))

            if L == 0 and 'va' in tap_d:
                nc.sync.dma_start(tap_d['va'][:], va)
            if L == 0 and 'q0' in tap_d:
                nc.sync.dma_start(tap_d['q0'][:], q_t[0])
            if L == 0 and 'k0' in tap_d:
                nc.sync.dma_start(tap_d['k0'][:], k_t[0])
            if L == 0 and 'vtot' in tap_d:
                nc.sync.dma_start(tap_d['vtot'][:], vtot_sb)
            # ---------- attention (shifted key blocks) ----------
            # software-pipelined: AV of head h-1 is emitted after the scores
            # of head h, so the PE queue never head-blocks on exp/mask
            a_tok = atp.tile([P, H, D], bf16, tag="at")

            def scores_block(h):
                hko = h // 2
                hr = slice(64 * (h % 2), 64 * (h % 2) + 64)
                pc = pcp.tile([P, NKB, 256], bf16, tag="pc")
                for kb in range(NKB):
                    qlo = max(0, (kb - 1) * P)
                    qhi = min(S, (kb + 1) * P)
                    qw = qhi - qlo
                    ps = psS.tile([P, 256], f32, tag="s")
                    nc.tensor.matmul(
                        ps[:, :qw],
                        lhsT=k_t[hko][hr, kb * P:(kb + 1) * P],
                        rhs=q_t[hko][hr, qlo:qhi],
                        start=True, stop=True)
                    es = pp.tile([P, 256], bf16, tag="es")
                    nc.scalar.activation(out=es[:, :qw], in_=ps[:, :qw],
                                         func=AF.Exp, scale=SCALE)
                    mc0 = 128 if kb == 0 else 0
                    nc.vector.scalar_tensor_tensor(
                        out=pc[:, kb, :qw], in0=es[:, :qw], scalar=1.0,
                        in1=mask_sb[:, mc0:mc0 + qw],
                        op0=OP.subtract, op1=OP.mult)
                if L == 0 and h == 0 and 'pc0' in tap_d:
                    nc.sync.dma_start(tap_d['pc0'][:], pc)
                return pc

            def av_block(h, pc, post_qb=None):
                for qb in range(8):
                    pav = psV.tile([P, 65], f32, tag="av")
                    c0 = 0 if qb == 0 else 128
                    nc.tensor.matmul(pav, lhsT=pc[:, qb, c0:c0 + P],
                                     rhs=va[:, qb, h, :], start=True, stop=False)
                    nc.tensor.matmul(pav, lhsT=pc[:, qb + 1, 0:P],
                                     rhs=va[:, qb + 1, h, :], start=False,
                                     stop=False)
                    nc.tensor.matmul(pav, lhsT=ones_row,
                                     rhs=vtot_sb[0:1, h * 65:(h + 1) * 65],
                                     start=False, stop=True)
                    rc = small.tile([P, 1], f32, tag="rc")
                    nc.vector.reciprocal(out=rc, in_=pav[:, 64:65])
                    nc.vector.tensor_scalar_mul(
                        out=a_tok[:, qb, h * 64:(h + 1) * 64],
                        in0=pav[:, 0:64], scalar1=rc)
                    if post_qb is not None:
                        post_qb(qb)

            x1T = big.tile([P, KO, S], bf16, tag="big")

            def ln1_block(qb):
                xn = tmp.tile([P, D], bf16, tag="xn")
                layer_norm_apply(a_tok[:, qb, :], xn)
                transpose_to(xn, x1T, qb)

            prev = None
            for h in range(H):
                pc_h = scores_block(h)
                if prev is not None:
                    av_block(prev[0], prev[1])
                prev = (h, pc_h)
            rstds = small.tile([P, 8], f32, tag="r8")
            mus = small.tile([P, 8], f32, tag="m8")

            def ln1_stats(qb):
                st = small.tile([P, 6], f32, tag="st")
                mv = small.tile([P, 2], f32, tag="mv")
                nc.vector.bn_stats(out=st, in_=a_tok[:, qb, :])
                nc.vector.bn_aggr(out=mv, in_=st)
                nc.scalar.activation(out=rstds[:, qb:qb + 1], in_=mv[:, 1:2],
                                     func=AF.Sqrt, bias=eps_sb[:, 0:1])
                nc.vector.reciprocal(out=rstds[:, qb:qb + 1],
                                     in_=rstds[:, qb:qb + 1])
                nc.vector.tensor_scalar_add(out=mus[:, qb:qb + 1],
                                            in0=mv[:, 0:1], scalar1=0.0)

            av_block(prev[0], prev[1], post_qb=ln1_stats)
            for qb in range(8):
                xn = tmp.tile([P, D], bf16, tag="xn")
                nc.vector.tensor_scalar(
                    out=xn, in0=a_tok[:, qb, :],
                    scalar1=mus[:, qb:qb + 1], scalar2=rstds[:, qb:qb + 1],
                    op0=OP.subtract, op1=OP.mult)
                if affine:
                    nc.vector.tensor_tensor(out=xn, in0=xn, in1=g_bc,
                                            op=OP.mult)
                    nc.vector.tensor_tensor(out=xn, in0=xn, in1=b_bc,
                                            op=OP.add)
                transpose_to(xn, x1T, qb)

            if L == 0 and 'atok' in tap_d:
                nc.sync.dma_start(tap_d['atok'][:], a_tok)
            if L == 0 and 'x1T' in tap_d:
                nc.sync.dma_start(tap_d['x1T'][:], x1T)
            # ---------- FFN + residual + LN2 -> next xT ----------
            xT_next = big.tile([P, KO, S], bf16, tag="big")
            for tq in range(2):
                qs = slice(tq * 512, (tq + 1) * 512)
                hts = []
                for hc in range(HC):
                    ph = psP.tile([P, D], f32, tag="pj")
                    for ko in range(KO):
                        nc.tensor.matmul(
                            ph,
                            lhsT=fc1_sb[:, ko, hc * P:(hc + 1) * P],
                            rhs=x1T[:, ko, qs],
                            start=(ko == 0), stop=(ko == KO - 1))
                    ht = hp.tile([P, D], bf16, tag=f"h{hc}")
                    if hc % 2 == 0:
                        nc.scalar.activation(out=ht, in_=ph, func=AF.Relu,
                                             bias=fc1b_sb[:, hc:hc + 1])
                    else:
                        nc.vector.tensor_scalar(
                            out=ht, in0=ph, scalar1=fc1b_sb[:, hc:hc + 1],
                            scalar2=0.0, op0=OP.add, op1=OP.max)
                    hts.append(ht)
                for tb2 in range(4):
                    tb = tq * 4 + tb2
                    pf = psP.tile([P, D], f32, tag="pj")
                    # fc2 first (start=True on hc0 clears the bank's
                    # has_written flags bank-wide), then the residual x1
                    # (feature-major, un-transposed) accumulates via
                    # identity matmuls with start=False
                    for hc in range(HC):
                        nc.tensor.matmul(
                            pf, lhsT=hts[hc][:, tb2 * P:(tb2 + 1) * P],
                            rhs=fc2_sb[:, hc, :],
                            start=(hc == 0), stop=False)
                    for dc in range(KO):
                        nc.tensor.matmul(
                            pf[:, dc * P:(dc + 1) * P],
                            lhsT=x1T[:, dc, tb * P:(tb + 1) * P],
                            rhs=ident, start=False, stop=(dc == KO - 1))
                    f = tmp.tile([P, D], bf16, tag="xn")
                    nc.vector.scalar_tensor_tensor(
                        out=f, in0=pf, scalar=0.0, in1=fc2b_bc,
                        op0=OP.add, op1=OP.add)
                    if L == 0 and 'f0' in tap_d:
                        nc.sync.dma_start(
                            tap_d['f0'][:, tb * D:(tb + 1) * D], f)
                    xn2 = tmp.tile([P, D], bf16, tag="xn")
                    layer_norm_apply(f, xn2)
                    transpose_to(xn2, xT_next, tb)
            if L == 0 and 'xnext' in tap_d:
                nc.sync.dma_start(tap_d['xnext'][:], xT_next)
            xT = xT_next

        # ---------- final projection: out[r] = sum(xT * owT[r]) ----------
        # DVE elementwise product (bf16, 4x mode), PE column-sum via
        # ones-vector matmuls accumulating 8 chunks into [1,512], then a
        # single-lane DVE reduce per class.
        red1 = wpool.tile([1, C], f32, tag="red1")
        for r in range(C):
            pcs_ = psT.tile([1, D], f32, tag="vt")
            for ko in range(KO):
                sc = scr.tile([P, S], bf16, tag="sc")
                nc.vector.tensor_tensor(
                    out=sc, in0=xT[:, ko, :], in1=owt[r][:, ko, :], op=OP.mult)
                for hf in range(2):
                    nc.tensor.matmul(
                        pcs_, lhsT=ones_col,
                        rhs=sc[:, hf * 512:(hf + 1) * 512],
                        start=(ko == 0 and hf == 0),
                        stop=(ko == KO - 1 and hf == 1))
            osc = scr.tile([1, D], f32, tag="osc")
            nc.scalar.copy(out=osc, in_=pcs_)
            nc.vector.reduce_sum(out=red1[:, r:r + 1], in_=osc,
                                 axis=mybir.AxisListType.X)
        nc.sync.dma_start(out_d[:], red1)

    nc.compile()
    return nc


def _shuffle_pko(a, inner):
    """[D_out*?, inner] row-major -> [P, blocks*inner] partition-major."""
    n = a.shape[0] // P
    return np.ascontiguousarray(
        a.reshape(n, P, inner).transpose(1, 0, 2).reshape(P, n * inner))


def _prep(inputs):
    """Host-side input prep shared across cores."""
    import ml_dtypes
    bf = ml_dtypes.bfloat16
    emb = np.asarray(inputs['emb'], dtype=np.float32)
    idx = np.asarray(inputs['inputs'])
    pos = np.arange(S, dtype=np.float32)[:, None]
    div = np.exp(-np.log(10000.0) * np.arange(0, D, 2, dtype=np.float32) / D)
    ang = pos * div
    pe = np.zeros((S, D), dtype=np.float32)
    pe[:, 0::2] = np.sin(ang)
    pe[:, 1::2] = np.cos(ang)
    x0 = emb[idx] + pe[None]  # [B, S, D]

    # mask[p, c] = 1 iff p < c <= p+128 (shifted-block band), tiled twice so
    # batched kb-pair ops can use one contiguous [P,512] operand
    jj = np.arange(P)[:, None]
    cc = np.arange(256)[None, :]
    m1 = ((cc > jj) & (cc <= jj + P))
    mask = np.concatenate([m1, m1], axis=1).astype(bf)

    ln_g = np.asarray(inputs['ln_g'], dtype=np.float32)
    ln_b = np.asarray(inputs['ln_b'], dtype=np.float32)
    affine = not (np.all(ln_g == 1.0) and np.all(ln_b == 0.0))

    out_w = np.asarray(inputs['out_w'], dtype=np.float32)
    # owT[c][p, ko*S+n] = out_w[c, n*D + ko*128+p]
    owT = out_w.reshape(C, S, D).transpose(0, 2, 1)  # [C, D, S]
    owT = np.ascontiguousarray(
        owT.reshape(C, KO, P, S).transpose(0, 2, 1, 3)
        .reshape(C, P, KO * S)).astype(bf)

    common = {
        'wqT': _shuffle_pko(np.asarray(inputs['wq'], np.float32).T.astype(bf), D),
        'wkT': _shuffle_pko(np.asarray(inputs['wk'], np.float32).T.astype(bf), D),
        'wvT': _shuffle_pko(np.asarray(inputs['wv'], np.float32).T.astype(bf), D),
        'bq': np.ascontiguousarray(
            np.asarray(inputs['bq'], np.float32).reshape(KO, P).T),
        'bk': np.ascontiguousarray(
            np.asarray(inputs['bk'], np.float32).reshape(KO, P).T),
        'bv': np.ascontiguousarray(np.asarray(inputs['bv'], np.float32)),
        'fc1T': _shuffle_pko(
            np.asarray(inputs['fc1_w'], np.float32).T.astype(bf), HD),
        'fc1b': np.ascontiguousarray(
            np.asarray(inputs['fc1_b'], np.float32).reshape(HC, P).T),
        'fc2T': _shuffle_pko(
            np.asarray(inputs['fc2_w'], np.float32).T.astype(bf), D),
        'fc2b': np.ascontiguousarray(np.asarray(inputs['fc2_b'], np.float32)),
        'mask': mask,
        'ident': np.eye(P, dtype=bf),
        'owT': owT,
    }
    if affine:
        common['lng'] = np.ascontiguousarray(ln_g)
        common['lnb'] = np.ascontiguousarray(ln_b)
    per_core = [
        {'xT': _shuffle_pko(x0[b].T.astype(bf), S)}
        for b in range(B)
    ]
    return common, per_core, affine


def kernel(**inputs):
    global LAST_EXEC_NS, LAST_RESULTS
    from concourse.bass_utils import run_bass_kernel_spmd

    common, per_core, affine = _prep(inputs)
    if affine not in _CACHE:
        _CACHE[affine] = _build(affine)
    nc = _CACHE[affine]

    in_maps = [dict(common, **pc) for pc in per_core]
    res = run_bass_kernel_spmd(nc, in_maps, list(range(B)), trace=TRACE)
    LAST_EXEC_NS = res.exec_time_ns
    LAST_RESULTS = res
    out = np.stack([res.results[b]["out"][0] for b in range(B)], axis=0)
    out = out + np.asarray(inputs['out_b'], np.float32)[None, :]
    return out.astype(np.float32)


# revision 38
# speedup vs baseline: 1.0229x; 1.0140x over previous
"""Trainium2 Bass kernel for nn_LocalModel (6-encoder local-attention transformer).

Sharding: data-parallel over batch — B=8 batch elements, one per NeuronCore.
Each core runs the full 6-layer encoder stack + final projection for its
batch element on-chip (all weights SBUF-resident in bf16), returning a
[6]-vector; the host gathers them into the [8, 6] output.

Attention uses the zero-masked-softmax identity: with out-of-window scores
set to 0 (not -inf), softmax over the full sequence satisfies
    out_i = (sum_{j in W} (e^{s_ij} - 1) v_j + sum_all v_j)
          / (sum_{j in W} (e^{s_ij} - 1) + S)

Key structure choices vs the f32 baseline:
  * everything bf16 except PSUM accumulation, LN stats and biases
  * key blocks are shifted by -W (=-64): block kb covers keys
    [kb*128-64, kb*128+64), so each 128-query block needs exactly 2 key
    blocks and every score tile is a uniform 256-wide band with ONE mask
    pattern; out-of-range keys are handled by zero-padded K columns
  * the "+sum_all v / +S" terms enter the AV PSUM accumulation as a K=1
    rank-1 matmul against an augmented V-total row (ones column = S)
  * (e^s - 1)*mask is a single fused DVE op (scalar_tensor_tensor)
  * the FFN residual is folded into the fc2 PSUM group via identity matmuls
  * LN rstd = exp(-0.5*ln(var+eps)) so every ACT func (Exp/Ln/Identity/
    Copy/Relu) lives in one activation table -> zero table reloads
  * out_w is prefetched in bf16 at kernel start across 3 DMA queues with a
    partition-contiguous layout; final dot products are fused DVE
    mult+accum ops
"""
import sys
import numpy as np

sys.path.insert(0, "/opt/trn_rl_repo")

B, S, D = 8, 1024, 512
H, Dh, W = 8, 64, 64
HD = 2048           # ffn hidden
C = 6               # classes
ENC = 6
EPS = 1e-5
P = 128
KO = D // P         # 4
HC = HD // P        # 16
NKB = 9             # shifted key blocks
SCALE = Dh ** -0.5

_CACHE = {}
LAST_EXEC_NS = None
LAST_RESULTS = None
TRACE = False


def _build(affine: bool, n_layers: int = ENC, taps: tuple = ()):
    import concourse.bass as bass
    import concourse.tile as tile
    from concourse import bacc, mybir
    from concourse.masks import make_identity

    f32 = mybir.dt.float32
    bf16 = mybir.dt.bfloat16
    AF = mybir.ActivationFunctionType
    OP = mybir.AluOpType

    nc = bacc.Bacc()
    d = {}
    d['xT'] = nc.declare_dram_parameter("xT", [P, KO * S], bf16, isOutput=False)
    for w in ("wqT", "wkT", "wvT"):
        d[w] = nc.declare_dram_parameter(w, [P, KO * D], bf16, isOutput=False)
    for b_ in ("bq", "bk"):
        d[b_] = nc.declare_dram_parameter(b_, [P, KO], f32, isOutput=False)
    d['bv'] = nc.declare_dram_parameter("bv", [D], f32, isOutput=False)
    d['fc1T'] = nc.declare_dram_parameter("fc1T", [P, KO * HD], bf16, isOutput=False)
    d['fc1b'] = nc.declare_dram_parameter("fc1b", [P, HC], f32, isOutput=False)
    d['fc2T'] = nc.declare_dram_parameter("fc2T", [P, HC * D], bf16, isOutput=False)
    d['fc2b'] = nc.declare_dram_parameter("fc2b", [D], f32, isOutput=False)
    d['mask'] = nc.declare_dram_parameter("mask", [P, 512], bf16, isOutput=False)
    d['ident'] = nc.declare_dram_parameter("ident", [P, P], bf16, isOutput=False)
    d['owT'] = nc.declare_dram_parameter("owT", [C, P, KO * S], bf16, isOutput=False)
    if affine:
        d['lng'] = nc.declare_dram_parameter("lng", [D], f32, isOutput=False)
        d['lnb'] = nc.declare_dram_parameter("lnb", [D], f32, isOutput=False)
    out_d = nc.declare_dram_parameter("out", [1, C], f32, isOutput=True)
    tap_d = {}
    for t in taps:
        shapes = {'va': [P, NKB * H * 65], 'q0': [P, S], 'k0': [P, W + S + W],
                  'vtot': [1, H * 65], 'pc0': [P, NKB * 256],
                  'atok': [P, H * D], 'x1T': [P, KO * S],
                  'f0': [P, 8 * D],
                  'xnext': [P, KO * S]}
        tap_d[t] = nc.declare_dram_parameter(
            "tap_" + t, shapes[t], bf16, isOutput=True)

    def bcast_ap(dram_h, parts=P):
        # replicate a [N] dram vector across `parts` partitions
        a = dram_h[:]
        return bass.AP(tensor=a.tensor, offset=a.offset,
                       ap=[[0, parts]] + [list(x) for x in a.ap])

    from contextlib import ExitStack
    with tile.TileContext(nc) as tc, ExitStack() as ctx:
        wpool = ctx.enter_context(tc.tile_pool(name="wpool", bufs=1))
        big = ctx.enter_context(tc.tile_pool(name="big", bufs=3))
        qkp = ctx.enter_context(tc.tile_pool(name="qkp", bufs=1))
        vap = ctx.enter_context(tc.tile_pool(name="vap", bufs=1))
        pcp = ctx.enter_context(tc.tile_pool(name="pcp", bufs=3))
        pp = ctx.enter_context(tc.tile_pool(name="pp", bufs=6))
        atp = ctx.enter_context(tc.tile_pool(name="atp", bufs=1))
        hp = ctx.enter_context(tc.tile_pool(name="hp", bufs=1))
        tmp = ctx.enter_context(tc.tile_pool(name="tmp", bufs=4))
        small = ctx.enter_context(tc.tile_pool(name="small", bufs=8))
        scr = ctx.enter_context(tc.tile_pool(name="scr", bufs=2))
        psP = ctx.enter_context(tc.tile_pool(name="psP", bufs=3, space="PSUM"))
        psS = ctx.enter_context(tc.tile_pool(name="psS", bufs=2, space="PSUM"))
        psV = ctx.enter_context(tc.tile_pool(name="psV", bufs=2, space="PSUM"))
        psT = ctx.enter_context(tc.tile_pool(name="psT", bufs=1, space="PSUM"))

        ident = wpool.tile([P, P], bf16, tag="id")
        nc.sync.dma_start(ident, d['ident'][:])
        ones_row = wpool.tile([1, P], bf16, tag="onr")
        nc.vector.memset(ones_row, 1.0)
        ones_col = wpool.tile([P, 1], bf16, tag="onc")
        nc.vector.memset(ones_col, 1.0)
        # PE clock pre-warm: ~6us of dummy matmuls on resident data while the
        # input DMAs land, so the HAM clock-gate is at 2.4GHz for real work
        for _ in range(28):
            wu = psS.tile([P, 256], f32, tag="s")
            nc.tensor.matmul(wu[:, :P], lhsT=ident, rhs=ident,
                             start=True, stop=True)
            nc.tensor.matmul(wu[:, P:], lhsT=ident, rhs=ident,
                             start=True, stop=True)
        # ---- persistent loads (queues: sync + scalar HWDGE, gpsimd SWDGE) ----
        xT = big.tile([P, KO, S], bf16, tag="big")
        nc.sync.dma_start(xT[:, 0:2, :], d['xT'][:, 0:2 * S])
        nc.scalar.dma_start(xT[:, 2:4, :], d['xT'][:, 2 * S:4 * S])
        wq_sb = wpool.tile([P, KO, D], bf16, tag="wq")
        wk_sb = wpool.tile([P, KO, D], bf16, tag="wk")
        wv_sb = wpool.tile([P, KO, D], bf16, tag="wv")
        nc.sync.dma_start(wq_sb, d['wqT'][:])
        nc.scalar.dma_start(wk_sb, d['wkT'][:])
        nc.scalar.dma_start(wv_sb, d['wvT'][:])
        fc1_sb = wpool.tile([P, KO, HD], bf16, tag="fc1")
        nc.sync.dma_start(fc1_sb, d['fc1T'][:])
        fc2_sb = wpool.tile([P, HC, D], bf16, tag="fc2")
        nc.sync.dma_start(fc2_sb, d['fc2T'][:])
        bq_sb = wpool.tile([P, KO], f32, tag="bq")
        bk_sb = wpool.tile([P, KO], f32, tag="bk")
        nc.gpsimd.dma_start(bq_sb, d['bq'][:])
        nc.gpsimd.dma_start(bk_sb, d['bk'][:])
        bv_bc = wpool.tile([P, D], f32, tag="bv")
        nc.gpsimd.dma_start(out=bv_bc, in_=bcast_ap(d['bv']))
        fc1b_sb = wpool.tile([P, HC], f32, tag="fc1b")
        nc.gpsimd.dma_start(fc1b_sb, d['fc1b'][:])
        fc2b_bc = wpool.tile([P, D], f32, tag="fc2b")
        nc.gpsimd.dma_start(out=fc2b_bc, in_=bcast_ap(d['fc2b']))
        mask_sb = wpool.tile([P, 512], bf16, tag="mask")
        nc.gpsimd.dma_start(mask_sb, d['mask'][:])
        if affine:
            g_bc = wpool.tile([P, D], f32, tag="g")
            b_bc = wpool.tile([P, D], f32, tag="b")
            nc.gpsimd.dma_start(out=g_bc, in_=bcast_ap(d['lng']))
            nc.gpsimd.dma_start(out=b_bc, in_=bcast_ap(d['lnb']))
        # out_w prefetch: fills DMA queues while layers 1-2 compute
        owt = []
        qs_cycle = [nc.sync, nc.scalar]
        for c in range(C):
            t = wpool.tile([P, KO, S], bf16, tag=f"ow{c}")
            qs_cycle[c % 2].dma_start(t, d['owT'][c])
            owt.append(t)

        eps_sb = wpool.tile([P, 1], f32, tag="eps")
        nc.vector.memset(eps_sb, EPS)
        vtot_sb = wpool.tile([1, H * 65], bf16, tag="vtot")
        bv1k = wpool.tile([1, D], f32, tag="bv1k")
        nc.scalar.mul(out=bv1k, in_=bv_bc[0:1, :], mul=float(S))

        def layer_norm_apply(src_ap, out_tile):
            """LayerNorm src [P,512] -> out_tile [P,512] bf16 (token-major).
            rstd via exp(-0.5*ln(var+eps)) to stay in one ACT table."""
            st = small.tile([P, 6], f32, tag="st")
            mv = small.tile([P, 2], f32, tag="mv")
            nc.vector.bn_stats(out=st, in_=src_ap)
            nc.vector.bn_aggr(out=mv, in_=st)
            rstd = small.tile([P, 1], f32, tag="rs")
            nc.scalar.activation(out=rstd, in_=mv[:, 1:2], func=AF.Sqrt,
                                 bias=eps_sb[:, 0:1])
            nc.vector.reciprocal(out=rstd, in_=rstd)
            nc.vector.tensor_scalar(out=out_tile, in0=src_ap,
                                    scalar1=mv[:, 0:1], scalar2=rstd,
                                    op0=OP.subtract, op1=OP.mult)
            if affine:
                nc.vector.tensor_tensor(out=out_tile, in0=out_tile, in1=g_bc,
                                        op=OP.mult)
                nc.vector.tensor_tensor(out=out_tile, in0=out_tile, in1=b_bc,
                                        op=OP.add)

        def transpose_to(src_tile, dst_tile, tb):
            """src [P, 512] bf16 token-major block tb -> dst [P, KO, S]."""
            pt = psS.tile([P, D], bf16, tag="s")
            for dc in range(KO):
                nc.tensor.transpose(pt[:, dc * P:(dc + 1) * P],
                                    src_tile[:, dc * P:(dc + 1) * P], ident)
            for dc in range(KO):
                if dc < 2:
                    nc.scalar.copy(
                        out=dst_tile[:, dc, tb * P:(tb + 1) * P],
                        in_=pt[:, dc * P:(dc + 1) * P])
                else:
                    nc.vector.tensor_scalar_add(
                        out=dst_tile[:, dc, tb * P:(tb + 1) * P],
                        in0=pt[:, dc * P:(dc + 1) * P], scalar1=0.0)

        for L in range(n_layers):
            # ---------- Q/K projections (feature-major) ----------
            q_t, k_t = [], []
            for mc in range(KO):
                qm = qkp.tile([P, S], bf16, tag=f"q{mc}")
                km = qkp.tile([P, W + S + W], bf16, tag=f"k{mc}")
                q_t.append(qm)
                k_t.append(km)
                nc.vector.memset(km[:, 0:W], 0.0)
                nc.vector.memset(km[:, W + S:], 0.0)
                for half in range(2):
                    cs = slice(half * 512, (half + 1) * 512)
                    pq = psP.tile([P, D], f32, tag="pj")
                    for ko in range(KO):
                        nc.tensor.matmul(
                            pq, lhsT=wq_sb[:, ko, mc * P:(mc + 1) * P],
                            rhs=xT[:, ko, cs],
                            start=(ko == 0), stop=(ko == KO - 1))
                    nc.scalar.activation(out=qm[:, cs], in_=pq,
                                         func=AF.Identity, bias=bq_sb[:, mc:mc + 1])
                    pk = psP.tile([P, D], f32, tag="pj")
                    for ko in range(KO):
                        nc.tensor.matmul(
                            pk, lhsT=wk_sb[:, ko, mc * P:(mc + 1) * P],
                            rhs=xT[:, ko, cs],
                            start=(ko == 0), stop=(ko == KO - 1))
                    nc.vector.tensor_scalar_add(
                        out=km[:, W + half * 512:W + (half + 1) * 512], in0=pk,
                        scalar1=bk_sb[:, mc:mc + 1])

            # ---------- V totals part 1: xsum (DVE, overlaps QKV matmuls) ----
            xs32 = small.tile([P, KO], f32, tag="xs")
            for ko in range(KO):
                nc.vector.reduce_sum(out=xs32[:, ko:ko + 1], in_=xT[:, ko, :],
                                     axis=mybir.AxisListType.X)

            # ---------- V projection into shifted key blocks ----------
            va = vap.tile([P, NKB, H, 65], bf16, tag="va")
            # edge blocks: zero the never-written halves (incl. ones col),
            # then set all ones-columns
            nc.vector.memset(va[0:64, 0, :, :], 0.0)
            nc.vector.memset(va[64:P, NKB - 1, :, :], 0.0)
            nc.vector.memset(va[:, :, :, 64:65], 1.0)
            for kb in range(NKB):
                tok0 = kb * P - W
                t0, t1 = max(0, tok0), min(S, tok0 + P)
                po, width = t0 - tok0, t1 - t0
                pv = psP.tile([P, D], f32, tag="pj")
                for ko in range(KO):
                    nc.tensor.matmul(
                        pv[po:po + width, :], lhsT=xT[:, ko, t0:t1],
                        rhs=wv_sb[:, ko, :],
                        start=(ko == 0), stop=(ko == KO - 1))
                nc.vector.tensor_tensor(
                    out=va[po:po + width, kb, :, 0:64],
                    in0=pv[po:po + width, :].rearrange("p (h a) -> p h a", a=64),
                    in1=bv_bc[po:po + width, :].rearrange("p (h a) -> p h a", a=64),
                    op=OP.add)

            # ---------- V totals part 2: (sum_t x) @ wvT + S*bv ----------
            xsr = small.tile([P, KO], bf16, tag="xsr")
            nc.scalar.copy(out=xsr, in_=xs32)
            pvt = psT.tile([1, D], f32, tag="vt")
            for ko in range(KO):
                nc.tensor.matmul(pvt, lhsT=xsr[:, ko:ko + 1],
                                 rhs=wv_sb[:, ko, :],
                                 start=(ko == 0), stop=(ko == KO - 1))
            nc.vector.tensor_tensor(
                out=vtot_sb.rearrange("p (h a) -> p h a", a=65)[:, :, 0:64],
                in0=pvt.rearrange("p (h a) -> p h a", a=64),
                in1=bv1k.rearrange("p (h a) -> p h a", a=64), op=OP.add)
            nc.vector.memset(
                vtot_sb.rearrange("p (h a) -> p h a", a=65)[:, :, 64:65],
                float(S))

            if L == 0 and 'va' in tap_d:
                nc.sync.dma_start(tap_d['va'][:], va)
            if L == 0 and 'q0' in tap_d:
                nc.sync.dma_start(tap_d['q0'][:], q_t[0])
            if L == 0 and 'k0' in tap_d:
                nc.sync.dma_start(tap_d['k0'][:], k_t[0])
            if L == 0 and 'vtot' in tap_d:
                nc.sync.dma_start(tap_d['vtot'][:], vtot_sb)
            # ---------- attention (shifted key blocks) ----------
            # software-pipelined: AV of head h-1 is emitted after the scores
            # of head h, so the PE queue never head-blocks on exp/mask
            a_tok = atp.tile([P, H, D], bf16, tag="at")

            def scores_block(h):
                hko = h // 2
                hr = slice(64 * (h % 2), 64 * (h % 2) + 64)
                pc = pcp.tile([P, NKB, 256], bf16, tag="pc")
                for kb in range(NKB):
                    qlo = max(0, (kb - 1) * P)
                    qhi = min(S, (kb + 1) * P)
                    qw = qhi - qlo
                    ps = psS.tile([P, 256], f32, tag="s")
                    nc.tensor.matmul(
                        ps[:, :qw],
                        lhsT=k_t[hko][hr, kb * P:(kb + 1) * P],
                        rhs=q_t[hko][hr, qlo:qhi],
                        start=True, stop=True)
                    es = pp.tile([P, 256], bf16, tag="es")
                    nc.scalar.activation(out=es[:, :qw], in_=ps[:, :qw],
                                         func=AF.Exp, scale=SCALE)
                    mc0 = 128 if kb == 0 else 0
                    nc.vector.scalar_tensor_tensor(
                        out=pc[:, kb, :qw], in0=es[:, :qw], scalar=1.0,
                        in1=mask_sb[:, mc0:mc0 + qw],
                        op0=OP.subtract, op1=OP.mult)
                if L == 0 and h == 0 and 'pc0' in tap_d:
                    nc.sync.dma_start(tap_d['pc0'][:], pc)
                return pc

            def av_block(h, pc, post_qb=None):
                for qb in range(8):
                    pav = psV.tile([P, 65], f32, tag="av")
                    c0 = 0 if qb == 0 else 128
                    nc.tensor.matmul(pav, lhsT=pc[:, qb, c0:c0 + P],
                                     rhs=va[:, qb, h, :], start=True, stop=False)
                    nc.tensor.matmul(pav, lhsT=pc[:, qb + 1, 0:P],
                                     rhs=va[:, qb + 1, h, :], start=False,
                                     stop=False)
                    nc.tensor.matmul(pav, lhsT=ones_row,
                                     rhs=vtot_sb[0:1, h * 65:(h + 1) * 65],
                                     start=False, stop=True)
                    rc = small.tile([P, 1], f32, tag="rc")
                    nc.vector.reciprocal(out=rc, in_=pav[:, 64:65])
                    nc.vector.tensor_scalar_mul(
                        out=a_tok[:, qb, h * 64:(h + 1) * 64],
                        in0=pav[:, 0:64], scalar1=rc)
                    if post_qb is not None:
                        post_qb(qb)

            x1T = big.tile([P, KO, S], bf16, tag="big")

            def ln1_block(qb):
                xn = tmp.tile([P, D], bf16, tag="xn")
                layer_norm_apply(a_tok[:, qb, :], xn)
                transpose_to(xn, x1T, qb)

            prev = None
            for h in range(H):
                pc_h = scores_block(h)
                if prev is not None:
                    av_block(prev[0], prev[1])
                prev = (h, pc_h)
            av_block(prev[0], prev[1])
            for qb in range(8):
                ln1_block(qb)

            if L == 0 and 'atok' in tap_d:
                nc.sync.dma_start(tap_d['atok'][:], a_tok)
            if L == 0 and 'x1T' in tap_d:
                nc.sync.dma_start(tap_d['x1T'][:], x1T)
            # ---------- FFN + residual + LN2 -> next xT ----------
            xT_next = big.tile([P, KO, S], bf16, tag="big")
            for tq in range(2):
                qs = slice(tq * 512, (tq + 1) * 512)
                hts = []
                for hc in range(HC):
                    ph = psP.tile([P, D], f32, tag="pj")
                    for ko in range(KO):
                        nc.tensor.matmul(
                            ph,
                            lhsT=fc1_sb[:, ko, hc * P:(hc + 1) * P],
                            rhs=x1T[:, ko, qs],
                            start=(ko == 0), stop=(ko == KO - 1))
                    ht = hp.tile([P, D], bf16, tag=f"h{hc}")
                    if hc % 2 == 0:
                        nc.scalar.activation(out=ht, in_=ph, func=AF.Relu,
                                             bias=fc1b_sb[:, hc:hc + 1])
                    else:
                        nc.vector.tensor_scalar(
                            out=ht, in0=ph, scalar1=fc1b_sb[:, hc:hc + 1],
                            scalar2=0.0, op0=OP.add, op1=OP.max)
                    hts.append(ht)
                for tb2 in range(4):
                    tb = tq * 4 + tb2
                    pf = psP.tile([P, D], f32, tag="pj")
                    # fc2 first (start=True on hc0 clears the bank's
                    # has_written flags bank-wide), then the residual x1
                    # (feature-major, un-transposed) accumulates via
                    # identity matmuls with start=False
                    for hc in range(HC):
                        nc.tensor.matmul(
                            pf, lhsT=hts[hc][:, tb2 * P:(tb2 + 1) * P],
                            rhs=fc2_sb[:, hc, :],
                            start=(hc == 0), stop=False)
                    for dc in range(KO):
                        nc.tensor.matmul(
                            pf[:, dc * P:(dc + 1) * P],
                            lhsT=x1T[:, dc, tb * P:(tb + 1) * P],
                            rhs=ident, start=False, stop=(dc == KO - 1))
                    f = tmp.tile([P, D], bf16, tag="xn")
                    nc.vector.scalar_tensor_tensor(
                        out=f, in0=pf, scalar=0.0, in1=fc2b_bc,
                        op0=OP.add, op1=OP.add)
                    if L == 0 and 'f0' in tap_d:
                        nc.sync.dma_start(
                            tap_d['f0'][:, tb * D:(tb + 1) * D], f)
                    xn2 = tmp.tile([P, D], bf16, tag="xn")
                    layer_norm_apply(f, xn2)
                    transpose_to(xn2, xT_next, tb)
            if L == 0 and 'xnext' in tap_d:
                nc.sync.dma_start(tap_d['xnext'][:], xT_next)
            xT = xT_next

        # ---------- final projection: out[r] = sum(xT * owT[r]) ----------
        # DVE elementwise product (bf16, 4x mode), PE column-sum via
        # ones-vector matmuls accumulating 8 chunks into [1,512], then a
        # single-lane DVE reduce per class.
        red1 = wpool.tile([1, C], f32, tag="red1")
        for r in range(C):
            pcs_ = psT.tile([1, D], f32, tag="vt")
            for ko in range(KO):
                sc = scr.tile([P, S], bf16, tag="sc")
                nc.vector.tensor_tensor(
                    out=sc, in0=xT[:, ko, :], in1=owt[r][:, ko, :], op=OP.mult)
                for hf in range(2):
                    nc.tensor.matmul(
                        pcs_, lhsT=ones_col,
                        rhs=sc[:, hf * 512:(hf + 1) * 512],
                        start=(ko == 0 and hf == 0),
                        stop=(ko == KO - 1 and hf == 1))
            osc = scr.tile([1, D], f32, tag="osc")
            nc.scalar.copy(out=osc, in_=pcs_)
            nc.vector.reduce_sum(out=red1[:, r:r + 1], in_=osc,
                                 axis=mybir.AxisListType.X)
        nc.sync.dma_start(out_d[:], red1)

    nc.compile()
    return nc


def _shuffle_pko(a, inner):
    """[D_out*?, inner] row-major -> [P, blocks*inner] partition-major."""
    n = a.shape[0] // P
    return np.ascontiguousarray(
        a.reshape(n, P, inner).transpose(1, 0, 2).reshape(P, n * inner))


def _prep(inputs):
    """Host-side input prep shared across cores."""
    import ml_dtypes
    bf = ml_dtypes.bfloat16
    emb = np.asarray(inputs['emb'], dtype=np.float32)
    idx = np.asarray(inputs['inputs'])
    pos = np.arange(S, dtype=np.float32)[:, None]
    div = np.exp(-np.log(10000.0) * np.arange(0, D, 2, dtype=np.float32) / D)
    ang = pos * div
    pe = np.zeros((S, D), dtype=np.float32)
    pe[:, 0::2] = np.sin(ang)
    pe[:, 1::2] = np.cos(ang)
    x0 = emb[idx] + pe[None]  # [B, S, D]

    # mask[p, c] = 1 iff p < c <= p+128 (shifted-block band), tiled twice so
    # batched kb-pair ops can use one contiguous [P,512] operand
    jj = np.arange(P)[:, None]
    cc = np.arange(256)[None, :]
    m1 = ((cc > jj) & (cc <= jj + P))
    mask = np.concatenate([m1, m1], axis=1).astype(bf)

    ln_g = np.asarray(inputs['ln_g'], dtype=np.float32)
    ln_b = np.asarray(inputs['ln_b'], dtype=np.float32)
    affine = not (np.all(ln_g == 1.0) and np.all(ln_b == 0.0))

    out_w = np.asarray(inputs['out_w'], dtype=np.float32)
    # owT[c][p, ko*S+n] = out_w[c, n*D + ko*128+p]
    owT = out_w.reshape(C, S, D).transpose(0, 2, 1)  # [C, D, S]
    owT = np.ascontiguousarray(
        owT.reshape(C, KO, P, S).transpose(0, 2, 1, 3)
        .reshape(C, P, KO * S)).astype(bf)

    common = {
        'wqT': _shuffle_pko(np.asarray(inputs['wq'], np.float32).T.astype(bf), D),
        'wkT': _shuffle_pko(np.asarray(inputs['wk'], np.float32).T.astype(bf), D),
        'wvT': _shuffle_pko(np.asarray(inputs['wv'], np.float32).T.astype(bf), D),
        'bq': np.ascontiguousarray(
            np.asarray(inputs['bq'], np.float32).reshape(KO, P).T),
        'bk': np.ascontiguousarray(
            np.asarray(inputs['bk'], np.float32).reshape(KO, P).T),
        'bv': np.ascontiguousarray(np.asarray(inputs['bv'], np.float32)),
        'fc1T': _shuffle_pko(
            np.asarray(inputs['fc1_w'], np.float32).T.astype(bf), HD),
        'fc1b': np.ascontiguousarray(
            np.asarray(inputs['fc1_b'], np.float32).reshape(HC, P).T),
        'fc2T': _shuffle_pko(
            np.asarray(inputs['fc2_w'], np.float32).T.astype(bf), D),
        'fc2b': np.ascontiguousarray(np.asarray(inputs['fc2_b'], np.float32)),
        'mask': mask,
        'ident': np.eye(P, dtype=bf),
        'owT': owT,
    }
    if affine:
        common['lng'] = np.ascontiguousarray(ln_g)
        common['lnb'] = np.ascontiguousarray(ln_b)
    per_core = [
        {'xT': _shuffle_pko(x0[b].T.astype(bf), S)}
        for b in range(B)
    ]
    return common, per_core, affine


def kernel(**inputs):
    global LAST_EXEC_NS, LAST_RESULTS
    from concourse.bass_utils import run_bass_kernel_spmd

    common, per_core, affine = _prep(inputs)
    if affine not in _CACHE:
        _CACHE[affine] = _build(affine)
    nc = _CACHE[affine]

    in_maps = [dict(common, **pc) for pc in per_core]
    res = run_bass_kernel_spmd(nc, in_maps, list(range(B)), trace=TRACE)
    LAST_EXEC_NS = res.exec_time_ns
    LAST_RESULTS = res
    out = np.stack([res.results[b]["out"][0] for b in range(B)], axis=0)
    out = out + np.asarray(inputs['out_b'], np.float32)[None, :]
    return out.astype(np.float32)


# revision 39
# speedup vs baseline: 1.0443x; 1.0209x over previous
"""Trainium2 Bass kernel for nn_LocalModel (6-encoder local-attention transformer).

Sharding: data-parallel over batch — B=8 batch elements, one per NeuronCore.
Each core runs the full 6-layer encoder stack + final projection for its
batch element on-chip (all weights SBUF-resident in bf16), returning a
[6]-vector; the host gathers them into the [8, 6] output.

Attention uses the zero-masked-softmax identity: with out-of-window scores
set to 0 (not -inf), softmax over the full sequence satisfies
    out_i = (sum_{j in W} (e^{s_ij} - 1) v_j + sum_all v_j)
          / (sum_{j in W} (e^{s_ij} - 1) + S)

Key structure choices vs the f32 baseline:
  * everything bf16 except PSUM accumulation, LN stats and biases
  * key blocks are shifted by -W (=-64): block kb covers keys
    [kb*128-64, kb*128+64), so each 128-query block needs exactly 2 key
    blocks and every score tile is a uniform 256-wide band with ONE mask
    pattern; out-of-range keys are handled by zero-padded K columns
  * the "+sum_all v / +S" terms enter the AV PSUM accumulation as a K=1
    rank-1 matmul against an augmented V-total row (ones column = S)
  * (e^s - 1)*mask is a single fused DVE op (scalar_tensor_tensor)
  * the FFN residual is folded into the fc2 PSUM group via identity matmuls
  * LN rstd = exp(-0.5*ln(var+eps)) so every ACT func (Exp/Ln/Identity/
    Copy/Relu) lives in one activation table -> zero table reloads
  * out_w is prefetched in bf16 at kernel start across 3 DMA queues with a
    partition-contiguous layout; final dot products are fused DVE
    mult+accum ops
"""
import sys
import numpy as np

sys.path.insert(0, "/opt/trn_rl_repo")

B, S, D = 8, 1024, 512
H, Dh, W = 8, 64, 64
HD = 2048           # ffn hidden
C = 6               # classes
ENC = 6
EPS = 1e-5
P = 128
KO = D // P         # 4
HC = HD // P        # 16
NKB = 9             # shifted key blocks
SCALE = Dh ** -0.5

_CACHE = {}
LAST_EXEC_NS = None
LAST_RESULTS = None
TRACE = False


def _build(affine: bool, n_layers: int = ENC, taps: tuple = ()):
    import concourse.bass as bass
    import concourse.tile as tile
    from concourse import bacc, mybir
    from concourse.masks import make_identity

    f32 = mybir.dt.float32
    bf16 = mybir.dt.bfloat16
    AF = mybir.ActivationFunctionType
    OP = mybir.AluOpType

    nc = bacc.Bacc()
    d = {}
    d['xT'] = nc.declare_dram_parameter("xT", [P, KO * S], bf16, isOutput=False)
    for w in ("wqT", "wkT", "wvT"):
        d[w] = nc.declare_dram_parameter(w, [P, KO * D], bf16, isOutput=False)
    for b_ in ("bq", "bk"):
        d[b_] = nc.declare_dram_parameter(b_, [P, KO], f32, isOutput=False)
    d['bv'] = nc.declare_dram_parameter("bv", [D], f32, isOutput=False)
    d['fc1T'] = nc.declare_dram_parameter("fc1T", [P, KO * HD], bf16, isOutput=False)
    d['fc1b'] = nc.declare_dram_parameter("fc1b", [P, HC], f32, isOutput=False)
    d['fc2T'] = nc.declare_dram_parameter("fc2T", [P, HC * D], bf16, isOutput=False)
    d['fc2b'] = nc.declare_dram_parameter("fc2b", [D], f32, isOutput=False)
    d['mask'] = nc.declare_dram_parameter("mask", [P, 512], bf16, isOutput=False)
    d['ident'] = nc.declare_dram_parameter("ident", [P, P], bf16, isOutput=False)
    d['owT'] = nc.declare_dram_parameter("owT", [C, P, KO * S], bf16, isOutput=False)
    if affine:
        d['lng'] = nc.declare_dram_parameter("lng", [D], f32, isOutput=False)
        d['lnb'] = nc.declare_dram_parameter("lnb", [D], f32, isOutput=False)
    out_d = nc.declare_dram_parameter("out", [1, C], f32, isOutput=True)
    tap_d = {}
    for t in taps:
        shapes = {'va': [P, NKB * H * 65], 'q0': [P, S], 'k0': [P, W + S + W],
                  'vtot': [1, H * 65], 'pc0': [P, NKB * 256],
                  'atok': [P, H * D], 'x1T': [P, KO * S],
                  'f0': [P, 8 * D],
                  'xnext': [P, KO * S]}
        tap_d[t] = nc.declare_dram_parameter(
            "tap_" + t, shapes[t], bf16, isOutput=True)

    def bcast_ap(dram_h, parts=P):
        # replicate a [N] dram vector across `parts` partitions
        a = dram_h[:]
        return bass.AP(tensor=a.tensor, offset=a.offset,
                       ap=[[0, parts]] + [list(x) for x in a.ap])

    from contextlib import ExitStack
    with tile.TileContext(nc) as tc, ExitStack() as ctx:
        wpool = ctx.enter_context(tc.tile_pool(name="wpool", bufs=1))
        big = ctx.enter_context(tc.tile_pool(name="big", bufs=3))
        qkp = ctx.enter_context(tc.tile_pool(name="qkp", bufs=1))
        vap = ctx.enter_context(tc.tile_pool(name="vap", bufs=1))
        pcp = ctx.enter_context(tc.tile_pool(name="pcp", bufs=3))
        pp = ctx.enter_context(tc.tile_pool(name="pp", bufs=6))
        atp = ctx.enter_context(tc.tile_pool(name="atp", bufs=1))
        hp = ctx.enter_context(tc.tile_pool(name="hp", bufs=1))
        tmp = ctx.enter_context(tc.tile_pool(name="tmp", bufs=4))
        small = ctx.enter_context(tc.tile_pool(name="small", bufs=8))
        scr = ctx.enter_context(tc.tile_pool(name="scr", bufs=2))
        psP = ctx.enter_context(tc.tile_pool(name="psP", bufs=3, space="PSUM"))
        psS = ctx.enter_context(tc.tile_pool(name="psS", bufs=2, space="PSUM"))
        psV = ctx.enter_context(tc.tile_pool(name="psV", bufs=3, space="PSUM"))

        ident = wpool.tile([P, P], bf16, tag="id")
        nc.sync.dma_start(ident, d['ident'][:])
        ones_row = wpool.tile([1, P], bf16, tag="onr")
        nc.vector.memset(ones_row, 1.0)
        ones_col = wpool.tile([P, 1], bf16, tag="onc")
        nc.vector.memset(ones_col, 1.0)
        # PE clock pre-warm: ~6us of dummy matmuls on resident data while the
        # input DMAs land, so the HAM clock-gate is at 2.4GHz for real work
        for _ in range(28):
            wu = psS.tile([P, 256], f32, tag="s")
            nc.tensor.matmul(wu[:, :P], lhsT=ident, rhs=ident,
                             start=True, stop=True)
            nc.tensor.matmul(wu[:, P:], lhsT=ident, rhs=ident,
                             start=True, stop=True)
        # ---- persistent loads (queues: sync + scalar HWDGE, gpsimd SWDGE) ----
        xT = big.tile([P, KO, S], bf16, tag="big")
        nc.sync.dma_start(xT[:, 0:2, :], d['xT'][:, 0:2 * S])
        nc.scalar.dma_start(xT[:, 2:4, :], d['xT'][:, 2 * S:4 * S])
        wq_sb = wpool.tile([P, KO, D], bf16, tag="wq")
        wk_sb = wpool.tile([P, KO, D], bf16, tag="wk")
        wv_sb = wpool.tile([P, KO, D], bf16, tag="wv")
        nc.sync.dma_start(wq_sb, d['wqT'][:])
        nc.scalar.dma_start(wk_sb, d['wkT'][:])
        nc.scalar.dma_start(wv_sb, d['wvT'][:])
        fc1_sb = wpool.tile([P, KO, HD], bf16, tag="fc1")
        nc.sync.dma_start(fc1_sb, d['fc1T'][:])
        fc2_sb = wpool.tile([P, HC, D], bf16, tag="fc2")
        nc.sync.dma_start(fc2_sb, d['fc2T'][:])
        bq_sb = wpool.tile([P, KO], f32, tag="bq")
        bk_sb = wpool.tile([P, KO], f32, tag="bk")
        nc.gpsimd.dma_start(bq_sb, d['bq'][:])
        nc.gpsimd.dma_start(bk_sb, d['bk'][:])
        bv_bc = wpool.tile([P, D], f32, tag="bv")
        nc.gpsimd.dma_start(out=bv_bc, in_=bcast_ap(d['bv']))
        fc1b_sb = wpool.tile([P, HC], f32, tag="fc1b")
        nc.gpsimd.dma_start(fc1b_sb, d['fc1b'][:])
        fc2b_bc = wpool.tile([P, D], f32, tag="fc2b")
        nc.gpsimd.dma_start(out=fc2b_bc, in_=bcast_ap(d['fc2b']))
        mask_sb = wpool.tile([P, 512], bf16, tag="mask")
        nc.gpsimd.dma_start(mask_sb, d['mask'][:])
        if affine:
            g_bc = wpool.tile([P, D], f32, tag="g")
            b_bc = wpool.tile([P, D], f32, tag="b")
            nc.gpsimd.dma_start(out=g_bc, in_=bcast_ap(d['lng']))
            nc.gpsimd.dma_start(out=b_bc, in_=bcast_ap(d['lnb']))
        # out_w prefetch: fills DMA queues while layers 1-2 compute
        owt = []
        qs_cycle = [nc.sync, nc.scalar]
        for c in range(C):
            t = wpool.tile([P, KO, S], bf16, tag=f"ow{c}")
            qs_cycle[c % 2].dma_start(t, d['owT'][c])
            owt.append(t)

        eps_sb = wpool.tile([P, 1], f32, tag="eps")
        nc.vector.memset(eps_sb, EPS)
        vtot_sb = wpool.tile([1, H * 65], bf16, tag="vtot")
        bv1k = wpool.tile([1, D], f32, tag="bv1k")
        nc.scalar.mul(out=bv1k, in_=bv_bc[0:1, :], mul=float(S))

        def layer_norm_apply(src_ap, out_tile):
            """LayerNorm src [P,512] -> out_tile [P,512] bf16 (token-major).
            rstd via exp(-0.5*ln(var+eps)) to stay in one ACT table."""
            st = small.tile([P, 6], f32, tag="st")
            mv = small.tile([P, 2], f32, tag="mv")
            nc.vector.bn_stats(out=st, in_=src_ap)
            nc.vector.bn_aggr(out=mv, in_=st)
            rstd = small.tile([P, 1], f32, tag="rs")
            nc.scalar.activation(out=rstd, in_=mv[:, 1:2], func=AF.Sqrt,
                                 bias=eps_sb[:, 0:1])
            nc.vector.reciprocal(out=rstd, in_=rstd)
            nc.vector.tensor_scalar(out=out_tile, in0=src_ap,
                                    scalar1=mv[:, 0:1], scalar2=rstd,
                                    op0=OP.subtract, op1=OP.mult)
            if affine:
                nc.vector.tensor_tensor(out=out_tile, in0=out_tile, in1=g_bc,
                                        op=OP.mult)
                nc.vector.tensor_tensor(out=out_tile, in0=out_tile, in1=b_bc,
                                        op=OP.add)

        def transpose_to(src_tile, dst_tile, tb):
            """src [P, 512] bf16 token-major block tb -> dst [P, KO, S]."""
            pt = psS.tile([P, D], bf16, tag="s")
            for dc in range(KO):
                nc.tensor.transpose(pt[:, dc * P:(dc + 1) * P],
                                    src_tile[:, dc * P:(dc + 1) * P], ident)
            for dc in range(KO):
                if dc < 2:
                    nc.scalar.copy(
                        out=dst_tile[:, dc, tb * P:(tb + 1) * P],
                        in_=pt[:, dc * P:(dc + 1) * P])
                else:
                    nc.vector.tensor_scalar_add(
                        out=dst_tile[:, dc, tb * P:(tb + 1) * P],
                        in0=pt[:, dc * P:(dc + 1) * P], scalar1=0.0)

        for L in range(n_layers):
            # ---------- Q/K projections (feature-major) ----------
            q_t, k_t = [], []
            for mc in range(KO):
                qm = qkp.tile([P, S], bf16, tag=f"q{mc}")
                km = qkp.tile([P, W + S + W], bf16, tag=f"k{mc}")
                q_t.append(qm)
                k_t.append(km)
                nc.vector.memset(km[:, 0:W], 0.0)
                nc.vector.memset(km[:, W + S:], 0.0)
                for half in range(2):
                    cs = slice(half * 512, (half + 1) * 512)
                    pq = psP.tile([P, D], f32, tag="pj")
                    for ko in range(KO):
                        nc.tensor.matmul(
                            pq, lhsT=wq_sb[:, ko, mc * P:(mc + 1) * P],
                            rhs=xT[:, ko, cs],
                            start=(ko == 0), stop=(ko == KO - 1))
                    nc.scalar.activation(out=qm[:, cs], in_=pq,
                                         func=AF.Identity, bias=bq_sb[:, mc:mc + 1])
                    pk = psP.tile([P, D], f32, tag="pj")
                    for ko in range(KO):
                        nc.tensor.matmul(
                            pk, lhsT=wk_sb[:, ko, mc * P:(mc + 1) * P],
                            rhs=xT[:, ko, cs],
                            start=(ko == 0), stop=(ko == KO - 1))
                    nc.vector.tensor_scalar_add(
                        out=km[:, W + half * 512:W + (half + 1) * 512], in0=pk,
                        scalar1=bk_sb[:, mc:mc + 1])

            # ---------- V totals part 1: xsum (DVE, overlaps QKV matmuls) ----
            xs32 = small.tile([P, KO], f32, tag="xs")
            for ko in range(KO):
                nc.vector.reduce_sum(out=xs32[:, ko:ko + 1], in_=xT[:, ko, :],
                                     axis=mybir.AxisListType.X)

            # ---------- V projection into shifted key blocks ----------
            va = vap.tile([P, NKB, H, 65], bf16, tag="va")
            # edge blocks: zero the never-written halves (incl. ones col),
            # then set all ones-columns
            nc.vector.memset(va[0:64, 0, :, :], 0.0)
            nc.vector.memset(va[64:P, NKB - 1, :, :], 0.0)
            nc.vector.memset(va[:, :, :, 64:65], 1.0)
            for kb in range(NKB):
                tok0 = kb * P - W
                t0, t1 = max(0, tok0), min(S, tok0 + P)
                po, width = t0 - tok0, t1 - t0
                pv = psP.tile([P, D], f32, tag="pj")
                for ko in range(KO):
                    nc.tensor.matmul(
                        pv[po:po + width, :], lhsT=xT[:, ko, t0:t1],
                        rhs=wv_sb[:, ko, :],
                        start=(ko == 0), stop=(ko == KO - 1))
                nc.vector.tensor_tensor(
                    out=va[po:po + width, kb, :, 0:64],
                    in0=pv[po:po + width, :].rearrange("p (h a) -> p h a", a=64),
                    in1=bv_bc[po:po + width, :].rearrange("p (h a) -> p h a", a=64),
                    op=OP.add)

            # ---------- V totals part 2: (sum_t x) @ wvT + S*bv ----------
            xsr = small.tile([P, KO], bf16, tag="xsr")
            nc.scalar.copy(out=xsr, in_=xs32)
            pvt = psP.tile([1, D], f32, tag="pj")
            for ko in range(KO):
                nc.tensor.matmul(pvt, lhsT=xsr[:, ko:ko + 1],
                                 rhs=wv_sb[:, ko, :],
                                 start=(ko == 0), stop=(ko == KO - 1))
            nc.vector.tensor_tensor(
                out=vtot_sb.rearrange("p (h a) -> p h a", a=65)[:, :, 0:64],
                in0=pvt.rearrange("p (h a) -> p h a", a=64),
                in1=bv1k.rearrange("p (h a) -> p h a", a=64), op=OP.add)
            nc.vector.memset(
                vtot_sb.rearrange("p (h a) -> p h a", a=65)[:, :, 64:65],
                float(S))

            if L == 0 and 'va' in tap_d:
                nc.sync.dma_start(tap_d['va'][:], va)
            if L == 0 and 'q0' in tap_d:
                nc.sync.dma_start(tap_d['q0'][:], q_t[0])
            if L == 0 and 'k0' in tap_d:
                nc.sync.dma_start(tap_d['k0'][:], k_t[0])
            if L == 0 and 'vtot' in tap_d:
                nc.sync.dma_start(tap_d['vtot'][:], vtot_sb)
            # ---------- attention (shifted key blocks) ----------
            # software-pipelined: AV of head h-1 is emitted after the scores
            # of head h, so the PE queue never head-blocks on exp/mask
            a_tok = atp.tile([P, H, D], bf16, tag="at")

            def scores_block(h):
                hko = h // 2
                hr = slice(64 * (h % 2), 64 * (h % 2) + 64)
                pc = pcp.tile([P, NKB, 256], bf16, tag="pc")
                for kb in range(NKB):
                    qlo = max(0, (kb - 1) * P)
                    qhi = min(S, (kb + 1) * P)
                    qw = qhi - qlo
                    ps = psS.tile([P, 256], f32, tag="s")
                    nc.tensor.matmul(
                        ps[:, :qw],
                        lhsT=k_t[hko][hr, kb * P:(kb + 1) * P],
                        rhs=q_t[hko][hr, qlo:qhi],
                        start=True, stop=True)
                    es = pp.tile([P, 256], bf16, tag="es")
                    nc.scalar.activation(out=es[:, :qw], in_=ps[:, :qw],
                                         func=AF.Exp, scale=SCALE)
                    mc0 = 128 if kb == 0 else 0
                    nc.vector.scalar_tensor_tensor(
                        out=pc[:, kb, :qw], in0=es[:, :qw], scalar=1.0,
                        in1=mask_sb[:, mc0:mc0 + qw],
                        op0=OP.subtract, op1=OP.mult)
                if L == 0 and h == 0 and 'pc0' in tap_d:
                    nc.sync.dma_start(tap_d['pc0'][:], pc)
                return pc

            def av_block(h, pc, post_qb=None):
                for qb in range(8):
                    pav = psV.tile([P, 65], f32, tag="av")
                    c0 = 0 if qb == 0 else 128
                    nc.tensor.matmul(pav, lhsT=pc[:, qb, c0:c0 + P],
                                     rhs=va[:, qb, h, :], start=True, stop=False)
                    nc.tensor.matmul(pav, lhsT=pc[:, qb + 1, 0:P],
                                     rhs=va[:, qb + 1, h, :], start=False,
                                     stop=False)
                    nc.tensor.matmul(pav, lhsT=ones_row,
                                     rhs=vtot_sb[0:1, h * 65:(h + 1) * 65],
                                     start=False, stop=True)
                    rc = small.tile([P, 1], f32, tag="rc")
                    nc.vector.reciprocal(out=rc, in_=pav[:, 64:65])
                    nc.vector.tensor_scalar_mul(
                        out=a_tok[:, qb, h * 64:(h + 1) * 64],
                        in0=pav[:, 0:64], scalar1=rc)
                    if post_qb is not None:
                        post_qb(qb)

            x1T = big.tile([P, KO, S], bf16, tag="big")

            def ln1_block(qb):
                xn = tmp.tile([P, D], bf16, tag="xn")
                layer_norm_apply(a_tok[:, qb, :], xn)
                transpose_to(xn, x1T, qb)

            prev = None
            for h in range(H):
                pc_h = scores_block(h)
                if prev is not None:
                    av_block(prev[0], prev[1])
                prev = (h, pc_h)
            av_block(prev[0], prev[1])
            for qb in range(8):
                ln1_block(qb)

            if L == 0 and 'atok' in tap_d:
                nc.sync.dma_start(tap_d['atok'][:], a_tok)
            if L == 0 and 'x1T' in tap_d:
                nc.sync.dma_start(tap_d['x1T'][:], x1T)
            # ---------- FFN + residual + LN2 -> next xT ----------
            xT_next = big.tile([P, KO, S], bf16, tag="big")
            for tq in range(2):
                qs = slice(tq * 512, (tq + 1) * 512)
                hts = []
                for hc in range(HC):
                    ph = psP.tile([P, D], f32, tag="pj")
                    for ko in range(KO):
                        nc.tensor.matmul(
                            ph,
                            lhsT=fc1_sb[:, ko, hc * P:(hc + 1) * P],
                            rhs=x1T[:, ko, qs],
                            start=(ko == 0), stop=(ko == KO - 1))
                    ht = hp.tile([P, D], bf16, tag=f"h{hc}")
                    if hc % 2 == 0:
                        nc.scalar.activation(out=ht, in_=ph, func=AF.Relu,
                                             bias=fc1b_sb[:, hc:hc + 1])
                    else:
                        nc.vector.tensor_scalar(
                            out=ht, in0=ph, scalar1=fc1b_sb[:, hc:hc + 1],
                            scalar2=0.0, op0=OP.add, op1=OP.max)
                    hts.append(ht)
                for tb2 in range(4):
                    tb = tq * 4 + tb2
                    pf = psP.tile([P, D], f32, tag="pj")
                    # fc2 first (start=True on hc0 clears the bank's
                    # has_written flags bank-wide), then the residual x1
                    # (feature-major, un-transposed) accumulates via
                    # identity matmuls with start=False
                    for hc in range(HC):
                        nc.tensor.matmul(
                            pf, lhsT=hts[hc][:, tb2 * P:(tb2 + 1) * P],
                            rhs=fc2_sb[:, hc, :],
                            start=(hc == 0), stop=False)
                    for dc in range(KO):
                        nc.tensor.matmul(
                            pf[:, dc * P:(dc + 1) * P],
                            lhsT=x1T[:, dc, tb * P:(tb + 1) * P],
                            rhs=ident, start=False, stop=(dc == KO - 1))
                    f = tmp.tile([P, D], bf16, tag="xn")
                    nc.vector.scalar_tensor_tensor(
                        out=f, in0=pf, scalar=0.0, in1=fc2b_bc,
                        op0=OP.add, op1=OP.add)
                    if L == 0 and 'f0' in tap_d:
                        nc.sync.dma_start(
                            tap_d['f0'][:, tb * D:(tb + 1) * D], f)
                    xn2 = tmp.tile([P, D], bf16, tag="xn")
                    layer_norm_apply(f, xn2)
                    transpose_to(xn2, xT_next, tb)
            if L == 0 and 'xnext' in tap_d:
                nc.sync.dma_start(tap_d['xnext'][:], xT_next)
            xT = xT_next

        # ---------- final projection: out[r] = sum(xT * owT[r]) ----------
        # DVE elementwise product (bf16, 4x mode), PE column-sum via
        # ones-vector matmuls accumulating 8 chunks into [1,512], then a
        # single-lane DVE reduce per class.
        red1 = wpool.tile([1, C], f32, tag="red1")
        for r in range(C):
            pcs_ = psP.tile([1, D], f32, tag="pj")
            for ko in range(KO):
                sc = scr.tile([P, S], bf16, tag="sc")
                nc.vector.tensor_tensor(
                    out=sc, in0=xT[:, ko, :], in1=owt[r][:, ko, :], op=OP.mult)
                for hf in range(2):
                    nc.tensor.matmul(
                        pcs_, lhsT=ones_col,
                        rhs=sc[:, hf * 512:(hf + 1) * 512],
                        start=(ko == 0 and hf == 0),
                        stop=(ko == KO - 1 and hf == 1))
            osc = scr.tile([1, D], f32, tag="osc")
            nc.scalar.copy(out=osc, in_=pcs_)
            nc.vector.reduce_sum(out=red1[:, r:r + 1], in_=osc,
                                 axis=mybir.AxisListType.X)
        nc.sync.dma_start(out_d[:], red1)

    nc.compile()
    return nc


def _shuffle_pko(a, inner):
    """[D_out*?, inner] row-major -> [P, blocks*inner] partition-major."""
    n = a.shape[0] // P
    return np.ascontiguousarray(
        a.reshape(n, P, inner).transpose(1, 0, 2).reshape(P, n * inner))


def _prep(inputs):
    """Host-side input prep shared across cores."""
    import ml_dtypes
    bf = ml_dtypes.bfloat16
    emb = np.asarray(inputs['emb'], dtype=np.float32)
    idx = np.asarray(inputs['inputs'])
    pos = np.arange(S, dtype=np.float32)[:, None]
    div = np.exp(-np.log(10000.0) * np.arange(0, D, 2, dtype=np.float32) / D)
    ang = pos * div
    pe = np.zeros((S, D), dtype=np.float32)
    pe[:, 0::2] = np.sin(ang)
    pe[:, 1::2] = np.cos(ang)
    x0 = emb[idx] + pe[None]  # [B, S, D]

    # mask[p, c] = 1 iff p < c <= p+128 (shifted-block band), tiled twice so
    # batched kb-pair ops can use one contiguous [P,512] operand
    jj = np.arange(P)[:, None]
    cc = np.arange(256)[None, :]
    m1 = ((cc > jj) & (cc <= jj + P))
    mask = np.concatenate([m1, m1], axis=1).astype(bf)

    ln_g = np.asarray(inputs['ln_g'], dtype=np.float32)
    ln_b = np.asarray(inputs['ln_b'], dtype=np.float32)
    affine = not (np.all(ln_g == 1.0) and np.all(ln_b == 0.0))

    out_w = np.asarray(inputs['out_w'], dtype=np.float32)
    # owT[c][p, ko*S+n] = out_w[c, n*D + ko*128+p]
    owT = out_w.reshape(C, S, D).transpose(0, 2, 1)  # [C, D, S]
    owT = np.ascontiguousarray(
        owT.reshape(C, KO, P, S).transpose(0, 2, 1, 3)
        .reshape(C, P, KO * S)).astype(bf)

    common = {
        'wqT': _shuffle_pko(np.asarray(inputs['wq'], np.float32).T.astype(bf), D),
        'wkT': _shuffle_pko(np.asarray(inputs['wk'], np.float32).T.astype(bf), D),
        'wvT': _shuffle_pko(np.asarray(inputs['wv'], np.float32).T.astype(bf), D),
        'bq': np.ascontiguousarray(
            np.asarray(inputs['bq'], np.float32).reshape(KO, P).T),
        'bk': np.ascontiguousarray(
            np.asarray(inputs['bk'], np.float32).reshape(KO, P).T),
        'bv': np.ascontiguousarray(np.asarray(inputs['bv'], np.float32)),
        'fc1T': _shuffle_pko(
            np.asarray(inputs['fc1_w'], np.float32).T.astype(bf), HD),
        'fc1b': np.ascontiguousarray(
            np.asarray(inputs['fc1_b'], np.float32).reshape(HC, P).T),
        'fc2T': _shuffle_pko(
            np.asarray(inputs['fc2_w'], np.float32).T.astype(bf), D),
        'fc2b': np.ascontiguousarray(np.asarray(inputs['fc2_b'], np.float32)),
        'mask': mask,
        'ident': np.eye(P, dtype=bf),
        'owT': owT,
    }
    if affine:
        common['lng'] = np.ascontiguousarray(ln_g)
        common['lnb'] = np.ascontiguousarray(ln_b)
    per_core = [
        {'xT': _shuffle_pko(x0[b].T.astype(bf), S)}
        for b in range(B)
    ]
    return common, per_core, affine


def kernel(**inputs):
    global LAST_EXEC_NS, LAST_RESULTS
    from concourse.bass_utils import run_bass_kernel_spmd

    common, per_core, affine = _prep(inputs)
    if affine not in _CACHE:
        _CACHE[affine] = _build(affine)
    nc = _CACHE[affine]

    in_maps = [dict(common, **pc) for pc in per_core]
    res = run_bass_kernel_spmd(nc, in_maps, list(range(B)), trace=TRACE)
    LAST_EXEC_NS = res.exec_time_ns
    LAST_RESULTS = res
    out = np.stack([res.results[b]["out"][0] for b in range(B)], axis=0)
    out = out + np.asarray(inputs['out_b'], np.float32)[None, :]
    return out.astype(np.float32)
